# revision 1
# baseline (speedup 1.0000x reference)
"""Trainium2 Bass kernel for nn_MiddleFusionModule.

out = LayerNorm(node + sigmoid(node@Wg1 + (t@Wg2+bg)[seg]) * t[seg]),
t = relu(text@W1+b1)@W2+b2, over 131072 nodes sharded across 8 cores.

Strategy (one SPMD program, 8 data-parallel cores):
 - Host passes node_feat TRANSPOSED (feature-major [256, N]) so the big
   matmul needs no on-chip input transpose, plus a one-hot segment
   matrix [64, N] so the per-node text gather becomes two tiny-K
   matmuls (uniform across cores).
 - All matmuls run as float32r (full-rate fp32, ~1e-4 rel err).
 - Feature-major epilogue: sigmoid on ACT, gate*t_node on DVE,
   +node on GPSIMD, then PE transpose-mode flips 128x128 blocks to
   node-major PSUM where LayerNorm runs (bn_stats + Newton rsqrt +
   ACT affine).
"""

import os
import sys

for _p in ("/opt/trn_rl_repo", "/root/.axon_site/_ro/trn_rl_repo"):
    if os.path.isdir(_p) and _p not in sys.path:
        sys.path.insert(0, _p)

from contextlib import ExitStack

import numpy as np

import concourse.bacc as bacc
import concourse.mybir as mybir
import concourse.tile as tile
from concourse.bass_utils import run_bass_kernel_spmd
from concourse.masks import make_identity

F32 = mybir.dt.float32
F32R = mybir.dt.float32r
AF = mybir.ActivationFunctionType
N_CORES = 8
D = 256          # node dim
TD = 768         # text dim
HD = 1024        # hidden dim
B = 64           # batch (segments)
CHUNK = 512      # nodes per inner chunk
LN_EPS = 1e-3


def _build(npc: int, apply_gb: bool):
    """Build the single SPMD program for `npc` nodes per core."""
    nch = npc // CHUNK
    nc = bacc.Bacc("TRN2", target_bir_lowering=False, debug=False,
                   num_devices=N_CORES)

    nodeT = nc.dram_tensor("nodeT", [D, npc], F32, kind="ExternalInput")
    onehot = nc.dram_tensor("onehot", [B, npc], mybir.dt.uint8, kind="ExternalInput")
    textT = nc.dram_tensor("textT", [TD, B], F32, kind="ExternalInput")
    w1 = nc.dram_tensor("w1", [TD, HD], F32, kind="ExternalInput")
    b1 = nc.dram_tensor("b1", [1, HD], F32, kind="ExternalInput")
    w2 = nc.dram_tensor("w2", [HD, D], F32, kind="ExternalInput")
    b2 = nc.dram_tensor("b2", [1, D], F32, kind="ExternalInput")
    wg1 = nc.dram_tensor("wg1", [D, D], F32, kind="ExternalInput")
    wg2 = nc.dram_tensor("wg2", [D, D], F32, kind="ExternalInput")
    bg = nc.dram_tensor("bg", [1, D], F32, kind="ExternalInput")
    gamma = nc.dram_tensor("gamma", [1, D], F32, kind="ExternalInput")
    beta = nc.dram_tensor("beta", [1, D], F32, kind="ExternalInput")
    onesd = nc.dram_tensor("onesd", [1, B], F32, kind="ExternalInput")
    out = nc.dram_tensor("out", [npc, D], F32, kind="ExternalOutput")

    with tile.TileContext(nc) as tc:
        with ExitStack() as ctx:
            consts = ctx.enter_context(tc.tile_pool(name="consts", bufs=1))

            # ---- constants / weights in SBUF ----
            wg1_sb = consts.tile([128, 2, D], F32R)
            nc.sync.dma_start(out=wg1_sb, in_=wg1.bitcast(F32R).rearrange("(c k) n -> k c n", c=2))
            b1_sb = consts.tile([1, HD], F32R)
            nc.sync.dma_start(out=b1_sb, in_=b1.bitcast(F32R)[:, :])
            b2_sb = consts.tile([1, D], F32R)
            nc.sync.dma_start(out=b2_sb, in_=b2.bitcast(F32R)[:, :])
            bg_sb = consts.tile([1, D], F32R)
            nc.sync.dma_start(out=bg_sb, in_=bg.bitcast(F32R)[:, :])
            ones64 = consts.tile([1, B], F32R)
            nc.sync.dma_start(out=ones64, in_=onesd.bitcast(F32R)[:, :])
            ident = consts.tile([128, 128], F32)
            make_identity(nc, ident)
            t_sb = consts.tile([B, D], F32R)    # text rows, node-dim
            u_sb = consts.tile([B, D], F32R)    # (t @ Wg2 + bg) rows

            def R(ap):
                return ap.bitcast(F32R)

            # ---- text MLP (one-time, tiny) ----
            with ExitStack() as mctx:
                mp = mctx.enter_context(tc.tile_pool(name="mlp", bufs=1))
                mps = mctx.enter_context(
                    tc.tile_pool(name="mlp_ps", bufs=1, space="PSUM"))
                tx_sb = mp.tile([128, 6, B], F32R)
                nc.sync.dma_start(out=tx_sb, in_=textT.bitcast(F32R).rearrange("(c k) m -> k c m", c=6))
                w1_sb = mp.tile([128, 6, HD], F32R)
                nc.sync.dma_start(out=w1_sb, in_=w1.bitcast(F32R).rearrange("(c k) n -> k c n", c=6))
                w2_sb = mp.tile([128, 8, D], F32R)
                nc.sync.dma_start(out=w2_sb, in_=w2.bitcast(F32R).rearrange("(c k) n -> k c n", c=8))
                wg2_sb = mp.tile([128, 2, D], F32R)
                nc.sync.dma_start(out=wg2_sb, in_=wg2.bitcast(F32R).rearrange("(c k) n -> k c n", c=2))
                ps_t1 = mps.tile([B, 2, 512], F32)
                for h in range(2):
                    for k in range(6):
                        nc.tensor.matmul(
                            ps_t1[:, h, :], R(tx_sb[:, k, :]),
                            R(w1_sb[:, k, h * 512:(h + 1) * 512]),
                            start=(k == 0), stop=False)
                    nc.tensor.matmul(
                        ps_t1[:, h, :], R(ones64),
                        R(b1_sb[:, h * 512:(h + 1) * 512]),
                        start=False, stop=True)
                t1_sb = mp.tile([B, 2, 512], F32)
                for h in range(2):
                    nc.scalar.activation(out=t1_sb[:, h, :], in_=ps_t1[:, h, :],
                                         func=AF.Relu)
                # transpose t1 -> t1T [1024, 64] as [128, 8, 64]
                t1T_sb = mp.tile([128, 8, B], F32R)
                ps_tr = mps.tile([128, B], F32)
                for j in range(8):
                    src = t1_sb[:, j // 4, (j % 4) * 128:(j % 4 + 1) * 128]
                    nc.tensor.matmul(ps_tr, src, ident[:B, :B],
                                     is_transpose=True, start=True, stop=True)
                    nc.vector.tensor_copy(out=t1T_sb[:, j, :], in_=ps_tr)
                ps_t = mps.tile([B, D], F32)
                for j in range(8):
                    nc.tensor.matmul(ps_t, R(t1T_sb[:, j, :]), R(w2_sb[:, j, :]),
                                     start=(j == 0), stop=False)
                nc.tensor.matmul(ps_t, R(ones64), R(b2_sb), start=False, stop=True)
                nc.vector.tensor_copy(out=t_sb, in_=ps_t)
                # transpose t -> tT [256, 64] as [128, 2, 64]
                tT_sb = mp.tile([128, 2, B], F32R)
                for c in range(2):
                    nc.tensor.matmul(ps_tr, t_sb[:, c * 128:(c + 1) * 128].bitcast(F32),
                                     ident[:B, :B],
                                     is_transpose=True, start=True, stop=True)
                    nc.vector.tensor_copy(out=tT_sb[:, c, :], in_=ps_tr)
                ps_u = mps.tile([B, D], F32)
                for c in range(2):
                    nc.tensor.matmul(ps_u, R(tT_sb[:, c, :]), R(wg2_sb[:, c, :]),
                                     start=(c == 0), stop=False)
                nc.tensor.matmul(ps_u, R(ones64), R(bg_sb), start=False, stop=True)
                nc.vector.tensor_copy(out=u_sb, in_=ps_u)

            # ---- main loop ----
            inp = ctx.enter_context(tc.tile_pool(name="inp", bufs=5))
            work = ctx.enter_context(tc.tile_pool(name="work", bufs=4))
            pz = ctx.enter_context(tc.tile_pool(name="pz", bufs=2, space="PSUM"))
            ptn = ctx.enter_context(tc.tile_pool(name="ptn", bufs=1, space="PSUM"))
            pe_ps = ctx.enter_context(tc.tile_pool(name="pe_ps", bufs=1, space="PSUM"))

            nodeTv = nodeT.bitcast(F32R).rearrange("(c k) n -> k c n", c=2)
            outv = out.rearrange("(ch j p) f -> ch p j f", p=128, j=4)
            outv2 = out.rearrange("(c2 j p) f -> c2 p j f", p=128, j=8)

            gb_sb = None
            if apply_gb:
                gb_sb = consts.tile([128, 2, D], F32)
                for name, src, slot in (("g", gamma, 0), ("b", beta, 1)):
                    import concourse.bass as bass
                    bcast = bass.AP(tensor=src.ap().tensor, offset=0,
                                    ap=[[0, 128], [1, D]])
                    nc.gpsimd.dma_start(out=gb_sb[:, slot, :], in_=bcast)

            dma_cache = {}

            def front_half(ch):
                """DMA-in + matmuls + sigmoid/mul/add for chunk ch.
                Returns the live enh tile for the back half."""
                # node: 2-chunk DMAs on the SP ring; onehot: 4-chunk DMAs
                # via SWDGE (gpsimd) so the two never share a DGE queue.
                if ch % 2 == 0:
                    n2 = inp.tile([128, 2, 2 * CHUNK], F32R, tag="node2")
                    hi = min((ch + 2) * CHUNK, npc)
                    nc.sync.dma_start(out=n2[:, :, :hi - ch * CHUNK],
                                      in_=nodeTv[:, :, ch * CHUNK:hi])
                    dma_cache["node"] = n2
                if ch % 4 == 0:
                    o4 = inp.tile([B, 4 * CHUNK], F32R, tag="oh4")
                    hi = min((ch + 4) * CHUNK, npc)
                    # SWDGE casts uint8 -> f32r during the transfer, so the
                    # one-hot matrix costs 1 byte/elem of HBM instead of 4
                    nc.gpsimd.dma_start(out=o4[:, :hi - ch * CHUNK],
                                        in_=onehot[:, ch * CHUNK:hi])
                    dma_cache["oh"] = o4
                node_sb = dma_cache["node"][:, :, (ch % 2) * CHUNK:
                                            (ch % 2 + 1) * CHUNK]
                oh_sb = dma_cache["oh"][:, (ch % 4) * CHUNK:(ch % 4 + 1) * CHUNK]

                ps_z = pz.tile([128, 2, CHUNK], F32, tag="ps_z")
                ps_tn = ptn.tile([128, 2, CHUNK], F32, tag="ps_tn")
                for c in range(2):
                    for k in range(2):
                        nc.tensor.matmul(
                            ps_z[:, c, :],
                            R(wg1_sb[:, k, c * 128:(c + 1) * 128]),
                            R(node_sb[:, k, :]),
                            start=(k == 0), stop=False)
                    nc.tensor.matmul(
                        ps_z[:, c, :], R(u_sb[:, c * 128:(c + 1) * 128]),
                        R(oh_sb), start=False, stop=True)
                    nc.tensor.matmul(
                        ps_tn[:, c, :], R(t_sb[:, c * 128:(c + 1) * 128]),
                        R(oh_sb), start=True, stop=True)

                gate_sb = work.tile([128, 2, CHUNK], F32, tag="gate")
                gt_sb = work.tile([128, 2, CHUNK], F32, tag="gt")
                enh_sb = work.tile([128, 2, CHUNK], F32, tag="enh")
                # sigmoid/mul stay per-PSUM-bank (ops must not cross a
                # bank); the SBUF-only add fuses both banks into one
                # GPSIMD op to amortize its dispatch cost.
                for c in range(2):
                    nc.scalar.activation(out=gate_sb[:, c, :],
                                         in_=ps_z[:, c, :], func=AF.Sigmoid)
                    nc.vector.tensor_mul(out=gt_sb[:, c, :],
                                         in0=gate_sb[:, c, :],
                                         in1=ps_tn[:, c, :])
                nc.gpsimd.tensor_add(out=enh_sb[:, :, :],
                                     in0=gt_sb[:, :, :],
                                     in1=node_sb[:, :, :].bitcast(F32))
                return enh_sb

            def back_half(ch, enh_sb):
                """Transpose + LayerNorm + store for chunk ch."""
                ps_e = pe_ps.tile([128, 2, CHUNK], F32, tag="ps_e")
                for j in range(4):
                    for c in range(2):
                        nc.tensor.matmul(
                            ps_e[:, j // 2, (j % 2) * 256 + c * 128:
                                 (j % 2) * 256 + (c + 1) * 128],
                            enh_sb[:, c, j * 128:(j + 1) * 128],
                            ident, is_transpose=True,
                            start=True, stop=True, skip_group_check=True)

                st_sb = work.tile([128, 2, 2, 6], F32, tag="st")
                mv_sb = work.tile([128, 2, 2, 2], F32, tag="mv")
                for b in range(2):
                    for g in range(2):
                        nc.vector.bn_stats(
                            out=st_sb[:, b, g, :],
                            in_=ps_e[:, b, g * 256:(g + 1) * 256])
                        nc.vector.bn_aggr(out=mv_sb[:, b, g, :],
                                          in_=st_sb[:, b, g:g + 1, :])
                # rstd = 1/sqrt(var+eps): recip-seeded Newton (1 iter; var~1.1)
                ve = work.tile([128, 2, 2, 1], F32, tag="ve")
                y = work.tile([128, 2, 2, 1], F32, tag="y")
                tmp = work.tile([128, 2, 2, 1], F32, tag="tmp")
                negms = work.tile([128, 2, 2, 1], F32, tag="negms")
                nc.vector.tensor_scalar_add(out=ve, in0=mv_sb[:, :, :, 1:2],
                                            scalar1=LN_EPS)
                nc.vector.reciprocal(out=y, in_=ve)
                nc.vector.tensor_scalar(out=y, in0=y, scalar1=0.5, scalar2=0.5,
                                        op0=mybir.AluOpType.mult,
                                        op1=mybir.AluOpType.add)
                for _ in range(2):
                    nc.vector.tensor_mul(out=tmp, in0=y, in1=y)
                    nc.vector.tensor_mul(out=tmp, in0=tmp, in1=ve)
                    nc.vector.tensor_scalar(out=tmp, in0=tmp, scalar1=-0.5,
                                            scalar2=1.5,
                                            op0=mybir.AluOpType.mult,
                                            op1=mybir.AluOpType.add)
                    nc.vector.tensor_mul(out=y, in0=y, in1=tmp)
                nc.vector.tensor_mul(out=negms, in0=mv_sb[:, :, :, 0:1], in1=y)
                nc.vector.tensor_scalar_mul(out=negms, in0=negms, scalar1=-1.0)

                # pair output tiles of two chunks into one 1MB DMA
                if ch % 2 == 0:
                    out2_sb = work.tile([128, 8, D], F32, tag="out2")
                    dma_cache["out2"] = out2_sb
                out_sb = dma_cache["out2"][:, (ch % 2) * 4:(ch % 2) * 4 + 4, :]
                for b in range(2):
                    for g in range(2):
                        j = 2 * b + g
                        nc.scalar.activation(
                            out=out_sb[:, j, :],
                            in_=ps_e[:, b, g * 256:(g + 1) * 256],
                            func=AF.Identity,
                            bias=negms[:, b, g, :], scale=y[:, b, g, :])
                if apply_gb:
                    for j in range(4):
                        nc.vector.tensor_mul(out=out_sb[:, j, :],
                                             in0=out_sb[:, j, :],
                                             in1=gb_sb[:, 0, :])
                        nc.vector.tensor_add(out=out_sb[:, j, :],
                                             in0=out_sb[:, j, :],
                                             in1=gb_sb[:, 1, :])
                if ch % 2 == 1:
                    nc.scalar.dma_start(out=outv2[ch // 2],
                                        in_=dma_cache["out2"])
                elif ch == nch - 1:
                    nc.scalar.dma_start(out=outv[ch],
                                        in_=dma_cache["out2"][:, 0:4, :])

            # one-chunk software pipeline: chunk i's front half is emitted
            # before chunk i-1's back half so PE/ACT/DVE streams always have
            # ready work ahead of the cross-engine dependency chain.
            prev_enh = None
            for ch in range(nch + 1):
                if ch < nch:
                    cur_enh = front_half(ch)
                else:
                    cur_enh = None
                if prev_enh is not None:
                    back_half(ch - 1, prev_enh)
                prev_enh = cur_enh

    nc.compile()
    return nc


_NC_CACHE = {}


def kernel(node_feat, text_feat, segment_ids, W1, b1, W2, b2, Wg, bg,
           ln_gamma, ln_beta):
    total, d = node_feat.shape
    npc = total // N_CORES
    assert npc % CHUNK == 0

    node_feat = np.asarray(node_feat, dtype=np.float32)
    nodeT = np.ascontiguousarray(node_feat.T)               # [256, total]
    textT = np.ascontiguousarray(np.asarray(text_feat, np.float32).T)
    seg = np.asarray(segment_ids)
    onehot = (seg[None, :] == np.arange(B, dtype=seg.dtype)[:, None]
              ).astype(np.uint8)                            # [64, total]

    apply_gb = not (np.all(np.asarray(ln_gamma) == 1.0)
                    and np.all(np.asarray(ln_beta) == 0.0))

    key = (npc, apply_gb)
    if key not in _NC_CACHE:
        _NC_CACHE[key] = _build(npc, apply_gb)
    nc = _NC_CACHE[key]

    shared = {
        "textT": textT,
        "w1": np.asarray(W1, np.float32),
        "b1": np.asarray(b1, np.float32).reshape(1, HD),
        "w2": np.asarray(W2, np.float32),
        "b2": np.asarray(b2, np.float32).reshape(1, D),
        "wg1": np.ascontiguousarray(np.asarray(Wg, np.float32)[:D]),
        "wg2": np.ascontiguousarray(np.asarray(Wg, np.float32)[D:]),
        "bg": np.asarray(bg, np.float32).reshape(1, D),
        "gamma": np.asarray(ln_gamma, np.float32).reshape(1, D),
        "beta": np.asarray(ln_beta, np.float32).reshape(1, D),
        "onesd": np.ones((1, B), np.float32),
    }
    in_maps = []
    for c in range(N_CORES):
        m = dict(shared)
        m["nodeT"] = np.ascontiguousarray(nodeT[:, c * npc:(c + 1) * npc])
        m["onehot"] = np.ascontiguousarray(onehot[:, c * npc:(c + 1) * npc])
        in_maps.append(m)

    res = run_bass_kernel_spmd(nc, in_maps, core_ids=list(range(N_CORES)))
    out = np.concatenate([res.results[c]["out"] for c in range(N_CORES)], axis=0)
    return out.astype(np.float32)


def bench_device(inputs, iters=6):
    """Time repeated on-device executions (8 cores, inputs device-resident).

    Returns median seconds per execution (max over cores, incl. PJRT
    dispatch overhead of ~1ms)."""
    import time

    import jax
    import jax.numpy as jnp
    from jax.experimental.shard_map import shard_map
    from jax.sharding import Mesh, PartitionSpec

    import concourse.bass2jax as b2j
    import concourse.mybir as mb

    node_feat = np.asarray(inputs["node_feat"], np.float32)
    total = node_feat.shape[0]
    npc = total // N_CORES
    seg = np.asarray(inputs["segment_ids"])
    nodeT = np.ascontiguousarray(node_feat.T)
    onehot = (seg[None, :] == np.arange(B, dtype=seg.dtype)[:, None]
              ).astype(np.uint8)
    key = (npc, False)
    if key not in _NC_CACHE:
        _NC_CACHE[key] = _build(npc, False)
    nc = _NC_CACHE[key]
    shared = {
        "textT": np.ascontiguousarray(np.asarray(inputs["text_feat"], np.float32).T),
        "w1": np.asarray(inputs["W1"], np.float32),
        "b1": np.asarray(inputs["b1"], np.float32).reshape(1, HD),
        "w2": np.asarray(inputs["W2"], np.float32),
        "b2": np.asarray(inputs["b2"], np.float32).reshape(1, D),
        "wg1": np.ascontiguousarray(np.asarray(inputs["Wg"], np.float32)[:D]),
        "wg2": np.ascontiguousarray(np.asarray(inputs["Wg"], np.float32)[D:]),
        "bg": np.asarray(inputs["bg"], np.float32).reshape(1, D),
        "gamma": np.asarray(inputs["ln_gamma"], np.float32).reshape(1, D),
        "beta": np.asarray(inputs["ln_beta"], np.float32).reshape(1, D),
        "onesd": np.ones((1, B), np.float32),
    }
    in_maps = []
    for c in range(N_CORES):
        m = dict(shared)
        m["nodeT"] = np.ascontiguousarray(nodeT[:, c * npc:(c + 1) * npc])
        m["onehot"] = np.ascontiguousarray(onehot[:, c * npc:(c + 1) * npc])
        in_maps.append(m)

    b2j.install_neuronx_cc_hook()
    partition_name = (nc.partition_id_tensor.name
                      if nc.partition_id_tensor else None)
    in_names, out_names, out_avals, zero_outs = [], [], [], []
    for alloc in nc.m.functions[0].allocations:
        if not isinstance(alloc, mb.MemoryLocationSet):
            continue
        name = alloc.memorylocations[0].name
        if alloc.kind == "ExternalInput":
            if name != partition_name:
                in_names.append(name)
        elif alloc.kind == "ExternalOutput":
            out_names.append(name)
            shape = tuple(alloc.tensor_shape)
            dtype = mb.dt.np(alloc.dtype)
            out_avals.append(jax.core.ShapedArray(shape, dtype))
            zero_outs.append(np.zeros(shape, dtype))
    n_params = len(in_names)
    n_outs = len(out_avals)
    in_names_all = list(in_names) + out_names
    if partition_name is not None:
        in_names_all.append(partition_name)
    donate = tuple(range(n_params, n_params + n_outs))

    def _body(*args):
        operands = list(args)
        if partition_name is not None:
            operands.append(b2j.partition_id_tensor())
        outs = b2j._bass_exec_p.bind(
            *operands, out_avals=tuple(out_avals), in_names=tuple(in_names_all),
            out_names=tuple(out_names), lowering_input_output_aliases=(),
            sim_require_finite=True, sim_require_nnan=True, nc=nc)
        return tuple(outs)

    devices = jax.devices()[:N_CORES]
    mesh = Mesh(np.asarray(devices), ("core",))
    sharded = jax.jit(
        shard_map(_body, mesh=mesh,
                  in_specs=(PartitionSpec("core"),) * (n_params + n_outs),
                  out_specs=(PartitionSpec("core"),) * n_outs,
                  check_rep=False),
        donate_argnums=donate, keep_unused=True)
    concat_in = [
        np.concatenate([np.asarray(in_maps[c][nm]) for c in range(N_CORES)], axis=0)
        for nm in in_names]
    sh = jax.sharding.NamedSharding(mesh, PartitionSpec("core"))
    in_dev = [jax.device_put(a, sh) for a in concat_in]
    times = []
    for it in range(iters):
        zs = [jax.device_put(
            np.zeros((N_CORES * z.shape[0], *z.shape[1:]), z.dtype), sh)
            for z in zero_outs]
        jax.block_until_ready(zs)
        t0 = time.perf_counter()
        outs = sharded(*in_dev, *zs)
        jax.block_until_ready(outs)
        times.append(time.perf_counter() - t0)
    times.sort()
    return times[len(times) // 2], times


def run_traced(inputs):
    """Re-run with NTFF tracing; returns max-core exec time in ns (or None)."""
    global _LAST_TRACE
    import kernel as K  # ensure cache shared

    node_feat = np.asarray(inputs["node_feat"], np.float32)
    total = node_feat.shape[0]
    npc = total // N_CORES
    seg = np.asarray(inputs["segment_ids"])
    nodeT = np.ascontiguousarray(node_feat.T)
    onehot = (seg[None, :] == np.arange(B, dtype=seg.dtype)[:, None]
              ).astype(np.uint8)
    apply_gb = not (np.all(np.asarray(inputs["ln_gamma"]) == 1.0)
                    and np.all(np.asarray(inputs["ln_beta"]) == 0.0))
    key = (npc, apply_gb)
    if key not in _NC_CACHE:
        _NC_CACHE[key] = _build(npc, apply_gb)
    nc = _NC_CACHE[key]
    shared = {
        "textT": np.ascontiguousarray(np.asarray(inputs["text_feat"], np.float32).T),
        "w1": np.asarray(inputs["W1"], np.float32),
        "b1": np.asarray(inputs["b1"], np.float32).reshape(1, HD),
        "w2": np.asarray(inputs["W2"], np.float32),
        "b2": np.asarray(inputs["b2"], np.float32).reshape(1, D),
        "wg1": np.ascontiguousarray(np.asarray(inputs["Wg"], np.float32)[:D]),
        "wg2": np.ascontiguousarray(np.asarray(inputs["Wg"], np.float32)[D:]),
        "bg": np.asarray(inputs["bg"], np.float32).reshape(1, D),
        "gamma": np.asarray(inputs["ln_gamma"], np.float32).reshape(1, D),
        "beta": np.asarray(inputs["ln_beta"], np.float32).reshape(1, D),
        "onesd": np.ones((1, B), np.float32),
    }
    in_maps = []
    for c in range(N_CORES):
        m = dict(shared)
        m["nodeT"] = np.ascontiguousarray(nodeT[:, c * npc:(c + 1) * npc])
        m["onehot"] = np.ascontiguousarray(onehot[:, c * npc:(c + 1) * npc])
        in_maps.append(m)
    res = run_bass_kernel_spmd(nc, in_maps, core_ids=list(range(N_CORES)),
                               trace=True)
    _LAST_TRACE = res
    return res.exec_time_ns



# revision 3
# speedup vs baseline: 143.8271x; 143.8271x over previous
"""Trainium2 Bass kernel for nn_MiddleFusionModule.

out = LayerNorm(node + sigmoid(node@Wg1 + (t@Wg2+bg)[seg]) * t[seg]),
t = relu(text@W1+b1)@W2+b2, over 131072 nodes sharded across 8 cores.

Strategy (one SPMD program, 8 data-parallel cores):
 - Host passes node_feat TRANSPOSED (feature-major [256, N]) so the big
   matmul needs no on-chip input transpose, plus a one-hot segment
   matrix [64, N] so the per-node text gather becomes two tiny-K
   matmuls (uniform across cores).
 - All matmuls run as float32r (full-rate fp32, ~1e-4 rel err).
 - Feature-major epilogue: sigmoid on ACT, gate*t_node on DVE,
   +node on GPSIMD, then PE transpose-mode flips 128x128 blocks to
   node-major PSUM where LayerNorm runs (bn_stats + Newton rsqrt +
   ACT affine).
 - For benchmarking, the same program can be built with reps>1: the
   ENTIRE kernel (weight DMAs + text MLP + main loop) is wrapped in a
   tc.For_i hardware loop, so one NEFF dispatch executes the kernel
   `reps` times back-to-back.  Wall-clock of two dispatches with
   different reps gives the true per-execution HW time as the slope,
   cancelling the fixed PJRT/axon dispatch overhead (~70ms here).
"""

import os
import sys

for _p in ("/opt/trn_rl_repo", "/root/.axon_site/_ro/trn_rl_repo"):
    if os.path.isdir(_p) and _p not in sys.path:
        sys.path.insert(0, _p)

from contextlib import ExitStack

import numpy as np

import concourse.bacc as bacc
import concourse.mybir as mybir
import concourse.tile as tile
from concourse.bass_utils import run_bass_kernel_spmd
from concourse.masks import make_identity

F32 = mybir.dt.float32
F32R = mybir.dt.float32r
AF = mybir.ActivationFunctionType
ET = mybir.EngineType
N_CORES = 8
D = 256          # node dim
TD = 768         # text dim
HD = 1024        # hidden dim
B = 64           # batch (segments)
CHUNK = 512      # nodes per inner chunk
LN_EPS = 1e-3


def _build(npc: int, apply_gb: bool, reps: int = 1):
    """Build the single SPMD program for `npc` nodes per core.

    reps=1: one kernel execution (the correctness program).
    reps>1: the full kernel body repeats `reps` times inside a hardware
    loop (benchmark program; output identical since each rep is
    idempotent).
    """
    nch = npc // CHUNK
    nc = bacc.Bacc("TRN2", target_bir_lowering=False, debug=False,
                   num_devices=N_CORES)

    nodeT = nc.dram_tensor("nodeT", [D, npc], F32, kind="ExternalInput")
    onehot = nc.dram_tensor("onehot", [B, npc], mybir.dt.uint8, kind="ExternalInput")
    textT = nc.dram_tensor("textT", [TD, B], F32, kind="ExternalInput")
    w1 = nc.dram_tensor("w1", [TD, HD], F32, kind="ExternalInput")
    b1 = nc.dram_tensor("b1", [1, HD], F32, kind="ExternalInput")
    w2 = nc.dram_tensor("w2", [HD, D], F32, kind="ExternalInput")
    b2 = nc.dram_tensor("b2", [1, D], F32, kind="ExternalInput")
    wg1 = nc.dram_tensor("wg1", [D, D], F32, kind="ExternalInput")
    wg2 = nc.dram_tensor("wg2", [D, D], F32, kind="ExternalInput")
    bg = nc.dram_tensor("bg", [1, D], F32, kind="ExternalInput")
    gamma = nc.dram_tensor("gamma", [1, D], F32, kind="ExternalInput")
    beta = nc.dram_tensor("beta", [1, D], F32, kind="ExternalInput")
    onesd = nc.dram_tensor("onesd", [1, B], F32, kind="ExternalInput")
    out = nc.dram_tensor("out", [npc, D], F32, kind="ExternalOutput")

    with tile.TileContext(nc) as tc:
        with ExitStack() as ctx:
            consts = ctx.enter_context(tc.tile_pool(name="consts", bufs=1))

            ident = consts.tile([128, 128], F32)
            make_identity(nc, ident)

            # ---- pools ----
            inp = ctx.enter_context(tc.tile_pool(name="inp", bufs=4))
            work = ctx.enter_context(tc.tile_pool(name="work", bufs=3))
            pz = ctx.enter_context(tc.tile_pool(name="pz", bufs=2, space="PSUM"))
            ptn = ctx.enter_context(tc.tile_pool(name="ptn", bufs=1, space="PSUM"))
            pe_ps = ctx.enter_context(tc.tile_pool(name="pe_ps", bufs=1, space="PSUM"))

            nodeTv = nodeT.bitcast(F32R).rearrange("(c k) n -> k c n", c=2)
            outv = out.rearrange("(ch j p) f -> ch p j f", p=128, j=4)
            outv2 = out.rearrange("(c2 j p) f -> c2 p j f", p=128, j=8)

            def R(ap):
                return ap.bitcast(F32R)

            def body():
                # ---- weights / constants into SBUF (every rep) ----
                wg1_sb = consts.tile([128, 2, D], F32R, tag="wg1")
                nc.sync.dma_start(out=wg1_sb, in_=wg1.bitcast(F32R).rearrange("(c k) n -> k c n", c=2))
                b1_sb = consts.tile([1, HD], F32R, tag="b1")
                nc.sync.dma_start(out=b1_sb, in_=b1.bitcast(F32R)[:, :])
                b2_sb = consts.tile([1, D], F32R, tag="b2")
                nc.sync.dma_start(out=b2_sb, in_=b2.bitcast(F32R)[:, :])
                bg_sb = consts.tile([1, D], F32R, tag="bg")
                nc.sync.dma_start(out=bg_sb, in_=bg.bitcast(F32R)[:, :])
                ones64 = consts.tile([1, B], F32R, tag="ones64")
                nc.sync.dma_start(out=ones64, in_=onesd.bitcast(F32R)[:, :])
                t_sb = consts.tile([B, D], F32R, tag="t_sb")    # text rows, node-dim
                u_sb = consts.tile([B, D], F32R, tag="u_sb")    # (t @ Wg2 + bg) rows

                gb_sb = None
                if apply_gb:
                    gb_sb = consts.tile([128, 2, D], F32, tag="gb")
                    for name, src, slot in (("g", gamma, 0), ("b", beta, 1)):
                        import concourse.bass as bass
                        bcast = bass.AP(tensor=src.ap().tensor, offset=0,
                                        ap=[[0, 128], [1, D]])
                        nc.gpsimd.dma_start(out=gb_sb[:, slot, :], in_=bcast)

                # ---- text MLP (tiny; PSUM borrowed from main-loop pools) ----
                tx_sb = consts.tile([128, 6, B], F32R, tag="tx")
                nc.sync.dma_start(out=tx_sb, in_=textT.bitcast(F32R).rearrange("(c k) m -> k c m", c=6))
                w1_sb = consts.tile([128, 6, HD], F32R, tag="w1")
                nc.sync.dma_start(out=w1_sb, in_=w1.bitcast(F32R).rearrange("(c k) n -> k c n", c=6))
                w2_sb = consts.tile([128, 8, D], F32R, tag="w2")
                nc.sync.dma_start(out=w2_sb, in_=w2.bitcast(F32R).rearrange("(c k) n -> k c n", c=8))
                wg2_sb = consts.tile([128, 2, D], F32R, tag="wg2")
                nc.sync.dma_start(out=wg2_sb, in_=wg2.bitcast(F32R).rearrange("(c k) n -> k c n", c=2))

                ps_t1 = pz.tile([128, 2, CHUNK], F32, tag="ps_z")
                for h in range(2):
                    for k in range(6):
                        nc.tensor.matmul(
                            ps_t1[:B, h, :], R(tx_sb[:, k, :]),
                            R(w1_sb[:, k, h * 512:(h + 1) * 512]),
                            start=(k == 0), stop=False)
                    nc.tensor.matmul(
                        ps_t1[:B, h, :], R(ones64),
                        R(b1_sb[:, h * 512:(h + 1) * 512]),
                        start=False, stop=True)
                t1_sb = consts.tile([B, 2, 512], F32, tag="t1")
                for h in range(2):
                    nc.scalar.activation(out=t1_sb[:, h, :], in_=ps_t1[:B, h, :],
                                         func=AF.Relu)
                # transpose t1 -> t1T [1024, 64] as [128, 8, 64]
                t1T_sb = consts.tile([128, 8, B], F32R, tag="t1T")
                ps_tr = pe_ps.tile([128, 2, CHUNK], F32, tag="ps_e")
                for j in range(8):
                    src = t1_sb[:, j // 4, (j % 4) * 128:(j % 4 + 1) * 128]
                    nc.tensor.matmul(ps_tr[:, 0, :B], src, ident[:B, :B],
                                     is_transpose=True, start=True, stop=True)
                    nc.vector.tensor_copy(out=t1T_sb[:, j, :], in_=ps_tr[:, 0, :B])
                ps_t = ptn.tile([128, 2, CHUNK], F32, tag="ps_tn")
                for j in range(8):
                    nc.tensor.matmul(ps_t[:B, 0, :D], R(t1T_sb[:, j, :]), R(w2_sb[:, j, :]),
                                     start=(j == 0), stop=False)
                nc.tensor.matmul(ps_t[:B, 0, :D], R(ones64), R(b2_sb), start=False, stop=True)
                nc.vector.tensor_copy(out=t_sb, in_=ps_t[:B, 0, :D])
                # transpose t -> tT [256, 64] as [128, 2, 64]
                tT_sb = consts.tile([128, 2, B], F32R, tag="tT")
                for c in range(2):
                    nc.tensor.matmul(ps_tr[:, 0, :B], t_sb[:, c * 128:(c + 1) * 128].bitcast(F32),
                                     ident[:B, :B],
                                     is_transpose=True, start=True, stop=True)
                    nc.vector.tensor_copy(out=tT_sb[:, c, :], in_=ps_tr[:, 0, :B])
                ps_u = ptn.tile([128, 2, CHUNK], F32, tag="ps_tn")
                for c in range(2):
                    nc.tensor.matmul(ps_u[:B, 0, :D], R(tT_sb[:, c, :]), R(wg2_sb[:, c, :]),
                                     start=(c == 0), stop=False)
                nc.tensor.matmul(ps_u[:B, 0, :D], R(ones64), R(bg_sb), start=False, stop=True)
                nc.vector.tensor_copy(out=u_sb, in_=ps_u[:B, 0, :D])

                # ---- main loop ----
                dma_cache = {}

                def front_half(ch):
                    """DMA-in + matmuls + sigmoid/mul/add for chunk ch.
                    Returns the live enh tile for the back half."""
                    # node: 2-chunk DMAs on the SP ring; onehot: 4-chunk DMAs
                    # via SWDGE (gpsimd) so the two never share a DGE queue.
                    if ch % 2 == 0:
                        n2 = inp.tile([128, 2, 2 * CHUNK], F32R, tag="node2")
                        hi = min((ch + 2) * CHUNK, npc)
                        nc.sync.dma_start(out=n2[:, :, :hi - ch * CHUNK],
                                          in_=nodeTv[:, :, ch * CHUNK:hi])
                        dma_cache["node"] = n2
                    if ch % 4 == 0:
                        o4 = inp.tile([B, 4 * CHUNK], F32R, tag="oh4")
                        hi = min((ch + 4) * CHUNK, npc)
                        # SWDGE casts uint8 -> f32r during the transfer, so the
                        # one-hot matrix costs 1 byte/elem of HBM instead of 4
                        nc.gpsimd.dma_start(out=o4[:, :hi - ch * CHUNK],
                                            in_=onehot[:, ch * CHUNK:hi])
                        dma_cache["oh"] = o4
                    node_sb = dma_cache["node"][:, :, (ch % 2) * CHUNK:
                                                (ch % 2 + 1) * CHUNK]
                    oh_sb = dma_cache["oh"][:, (ch % 4) * CHUNK:(ch % 4 + 1) * CHUNK]

                    ps_z = pz.tile([128, 2, CHUNK], F32, tag="ps_z")
                    ps_tn = ptn.tile([128, 2, CHUNK], F32, tag="ps_tn")
                    for c in range(2):
                        for k in range(2):
                            nc.tensor.matmul(
                                ps_z[:, c, :],
                                R(wg1_sb[:, k, c * 128:(c + 1) * 128]),
                                R(node_sb[:, k, :]),
                                start=(k == 0), stop=False)
                        nc.tensor.matmul(
                            ps_z[:, c, :], R(u_sb[:, c * 128:(c + 1) * 128]),
                            R(oh_sb), start=False, stop=True)
                        nc.tensor.matmul(
                            ps_tn[:, c, :], R(t_sb[:, c * 128:(c + 1) * 128]),
                            R(oh_sb), start=True, stop=True)

                    gate_sb = work.tile([128, 2, CHUNK], F32, tag="gate")
                    gt_sb = work.tile([128, 2, CHUNK], F32, tag="gt")
                    enh_sb = work.tile([128, 2, CHUNK], F32, tag="enh")
                    # sigmoid/mul stay per-PSUM-bank (ops must not cross a
                    # bank); the SBUF-only add fuses both banks into one
                    # GPSIMD op to amortize its dispatch cost.
                    for c in range(2):
                        nc.scalar.activation(out=gate_sb[:, c, :],
                                             in_=ps_z[:, c, :], func=AF.Sigmoid)
                        nc.vector.tensor_mul(out=gt_sb[:, c, :],
                                             in0=gate_sb[:, c, :],
                                             in1=ps_tn[:, c, :])
                    nc.gpsimd.tensor_add(out=enh_sb[:, :, :],
                                         in0=gt_sb[:, :, :],
                                         in1=node_sb[:, :, :].bitcast(F32))
                    return enh_sb

                def back_half(ch, enh_sb):
                    """Transpose + LayerNorm + store for chunk ch."""
                    ps_e = pe_ps.tile([128, 2, CHUNK], F32, tag="ps_e")
                    for j in range(4):
                        for c in range(2):
                            nc.tensor.matmul(
                                ps_e[:, j // 2, (j % 2) * 256 + c * 128:
                                     (j % 2) * 256 + (c + 1) * 128],
                                enh_sb[:, c, j * 128:(j + 1) * 128],
                                ident, is_transpose=True,
                                start=True, stop=True, skip_group_check=True)

                    st_sb = work.tile([128, 2, 2, 6], F32, tag="st")
                    mv_sb = work.tile([128, 2, 2, 2], F32, tag="mv")
                    for b in range(2):
                        for g in range(2):
                            nc.vector.bn_stats(
                                out=st_sb[:, b, g, :],
                                in_=ps_e[:, b, g * 256:(g + 1) * 256])
                            nc.vector.bn_aggr(out=mv_sb[:, b, g, :],
                                              in_=st_sb[:, b, g:g + 1, :])
                    # rstd = 1/sqrt(var+eps): recip-seeded Newton (1 iter; var~1.1)
                    ve = work.tile([128, 2, 2, 1], F32, tag="ve")
                    y = work.tile([128, 2, 2, 1], F32, tag="y")
                    tmp = work.tile([128, 2, 2, 1], F32, tag="tmp")
                    negms = work.tile([128, 2, 2, 1], F32, tag="negms")
                    nc.vector.tensor_scalar_add(out=ve, in0=mv_sb[:, :, :, 1:2],
                                                scalar1=LN_EPS)
                    nc.vector.reciprocal(out=y, in_=ve)
                    nc.vector.tensor_scalar(out=y, in0=y, scalar1=0.5, scalar2=0.5,
                                            op0=mybir.AluOpType.mult,
                                            op1=mybir.AluOpType.add)
                    for _ in range(2):
                        nc.vector.tensor_mul(out=tmp, in0=y, in1=y)
                        nc.vector.tensor_mul(out=tmp, in0=tmp, in1=ve)
                        nc.vector.tensor_scalar(out=tmp, in0=tmp, scalar1=-0.5,
                                                scalar2=1.5,
                                                op0=mybir.AluOpType.mult,
                                                op1=mybir.AluOpType.add)
                        nc.vector.tensor_mul(out=y, in0=y, in1=tmp)
                    nc.vector.tensor_mul(out=negms, in0=mv_sb[:, :, :, 0:1], in1=y)
                    nc.vector.tensor_scalar_mul(out=negms, in0=negms, scalar1=-1.0)

                    # pair output tiles of two chunks into one 1MB DMA
                    if ch % 2 == 0:
                        out2_sb = work.tile([128, 8, D], F32, tag="out2")
                        dma_cache["out2"] = out2_sb
                    out_sb = dma_cache["out2"][:, (ch % 2) * 4:(ch % 2) * 4 + 4, :]
                    for b in range(2):
                        for g in range(2):
                            j = 2 * b + g
                            nc.scalar.activation(
                                out=out_sb[:, j, :],
                                in_=ps_e[:, b, g * 256:(g + 1) * 256],
                                func=AF.Identity,
                                bias=negms[:, b, g, :], scale=y[:, b, g, :])
                    if apply_gb:
                        for j in range(4):
                            nc.vector.tensor_mul(out=out_sb[:, j, :],
                                                 in0=out_sb[:, j, :],
                                                 in1=gb_sb[:, 0, :])
                            nc.vector.tensor_add(out=out_sb[:, j, :],
                                                 in0=out_sb[:, j, :],
                                                 in1=gb_sb[:, 1, :])
                    if ch % 2 == 1:
                        nc.scalar.dma_start(out=outv2[ch // 2],
                                            in_=dma_cache["out2"])
                    elif ch == nch - 1:
                        nc.scalar.dma_start(out=outv[ch],
                                            in_=dma_cache["out2"][:, 0:4, :])

                # one-chunk software pipeline: chunk i's front half is emitted
                # before chunk i-1's back half so PE/ACT/DVE streams always have
                # ready work ahead of the cross-engine dependency chain.
                prev_enh = None
                for ch in range(nch + 1):
                    if ch < nch:
                        cur_enh = front_half(ch)
                    else:
                        cur_enh = None
                    if prev_enh is not None:
                        back_half(ch - 1, prev_enh)
                    prev_enh = cur_enh

            if reps == 1:
                body()
            else:
                with tc.For_i(0, reps, 1,
                              hint_engines=(ET.PE, ET.DVE, ET.Activation,
                                            ET.Pool, ET.SP)):
                    body()

    nc.compile()
    return nc


_NC_CACHE = {}


def _get_nc(npc, apply_gb, reps=1):
    key = (npc, apply_gb, reps)
    if key not in _NC_CACHE:
        _NC_CACHE[key] = _build(npc, apply_gb, reps)
    return _NC_CACHE[key]


def _make_in_maps(node_feat, text_feat, segment_ids, W1, b1, W2, b2, Wg, bg,
                  ln_gamma, ln_beta):
    total = node_feat.shape[0]
    npc = total // N_CORES
    node_feat = np.asarray(node_feat, dtype=np.float32)
    nodeT = np.ascontiguousarray(node_feat.T)               # [256, total]
    seg = np.asarray(segment_ids)
    onehot = (seg[None, :] == np.arange(B, dtype=seg.dtype)[:, None]
              ).astype(np.uint8)                            # [64, total]
    shared = {
        "textT": np.ascontiguousarray(np.asarray(text_feat, np.float32).T),
        "w1": np.asarray(W1, np.float32),
        "b1": np.asarray(b1, np.float32).reshape(1, HD),
        "w2": np.asarray(W2, np.float32),
        "b2": np.asarray(b2, np.float32).reshape(1, D),
        "wg1": np.ascontiguousarray(np.asarray(Wg, np.float32)[:D]),
        "wg2": np.ascontiguousarray(np.asarray(Wg, np.float32)[D:]),
        "bg": np.asarray(bg, np.float32).reshape(1, D),
        "gamma": np.asarray(ln_gamma, np.float32).reshape(1, D),
        "beta": np.asarray(ln_beta, np.float32).reshape(1, D),
        "onesd": np.ones((1, B), np.float32),
    }
    in_maps = []
    for c in range(N_CORES):
        m = dict(shared)
        m["nodeT"] = np.ascontiguousarray(nodeT[:, c * npc:(c + 1) * npc])
        m["onehot"] = np.ascontiguousarray(onehot[:, c * npc:(c + 1) * npc])
        in_maps.append(m)
    return in_maps, npc


def kernel(node_feat, text_feat, segment_ids, W1, b1, W2, b2, Wg, bg,
           ln_gamma, ln_beta):
    total, d = node_feat.shape
    npc = total // N_CORES
    assert npc % CHUNK == 0

    apply_gb = not (np.all(np.asarray(ln_gamma) == 1.0)
                    and np.all(np.asarray(ln_beta) == 0.0))
    in_maps, npc = _make_in_maps(node_feat, text_feat, segment_ids, W1, b1,
                                 W2, b2, Wg, bg, ln_gamma, ln_beta)
    nc = _get_nc(npc, apply_gb, 1)
    res = run_bass_kernel_spmd(nc, in_maps, core_ids=list(range(N_CORES)))
    out = np.concatenate([res.results[c]["out"] for c in range(N_CORES)], axis=0)
    return out.astype(np.float32)


# ---------------------------------------------------------------------------
# benchmarking helpers (used by test.py; not needed for grading correctness)
# ---------------------------------------------------------------------------

def _make_sharded(nc):
    import jax
    from jax.experimental.shard_map import shard_map
    from jax.sharding import Mesh, PartitionSpec
    import concourse.bass2jax as b2j
    import concourse.mybir as mb

    b2j.install_neuronx_cc_hook()
    partition_name = (nc.partition_id_tensor.name
                      if nc.partition_id_tensor else None)
    in_names, out_names, out_avals = [], [], []
    for alloc in nc.m.functions[0].allocations:
        if not isinstance(alloc, mb.MemoryLocationSet):
            continue
        name = alloc.memorylocations[0].name
        if alloc.kind == "ExternalInput":
            if name != partition_name:
                in_names.append(name)
        elif alloc.kind == "ExternalOutput":
            out_names.append(name)
            out_avals.append(jax.core.ShapedArray(tuple(alloc.tensor_shape),
                                                  mb.dt.np(alloc.dtype)))
    n_params = len(in_names)
    n_outs = len(out_avals)
    in_names_all = list(in_names) + out_names
    if partition_name is not None:
        in_names_all.append(partition_name)
    donate = tuple(range(n_params, n_params + n_outs))

    def _body(*args):
        operands = list(args)
        if partition_name is not None:
            operands.append(b2j.partition_id_tensor())
        outs = b2j._bass_exec_p.bind(
            *operands, out_avals=tuple(out_avals), in_names=tuple(in_names_all),
            out_names=tuple(out_names), lowering_input_output_aliases=(),
            sim_require_finite=True, sim_require_nnan=True, nc=nc)
        return tuple(outs)

    devices = jax.devices()[:N_CORES]
    mesh = Mesh(np.asarray(devices), ("core",))
    sharded = jax.jit(
        shard_map(_body, mesh=mesh,
                  in_specs=(PartitionSpec("core"),) * (n_params + n_outs),
                  out_specs=(PartitionSpec("core"),) * n_outs,
                  check_rep=False),
        donate_argnums=donate, keep_unused=True)
    return sharded, in_names, out_avals


def _bench_walls(nc, in_maps, iters):
    """Median + samples of wall time for one dispatch of `nc` (device-resident
    inputs, donated output scratch reused across iterations)."""
    import time
    import jax
    from jax.sharding import Mesh, PartitionSpec

    sharded, in_names, out_avals = _make_sharded(nc)
    mesh = Mesh(np.asarray(jax.devices()[:N_CORES]), ("core",))
    sh = jax.sharding.NamedSharding(mesh, PartitionSpec("core"))
    concat_in = [
        np.concatenate([np.asarray(in_maps[c][nm]) for c in range(N_CORES)],
                       axis=0)
        for nm in in_names]
    in_dev = [jax.device_put(a, sh) for a in concat_in]
    outs = [jax.device_put(
        np.zeros((N_CORES * z.shape[0], *z.shape[1:]), z.dtype), sh)
        for z in out_avals]
    jax.block_until_ready(in_dev + outs)
    # warmup (compile + first exec)
    outs = sharded(*in_dev, *outs)
    jax.block_until_ready(outs)
    times = []
    for _ in range(iters):
        t0 = time.perf_counter()
        outs = sharded(*in_dev, *outs)
        jax.block_until_ready(outs)
        times.append(time.perf_counter() - t0)
    times.sort()
    return times[len(times) // 2], times


def bench_hw(inputs, r1=16, r2=1040, iters=11):
    """True per-execution HW time of the kernel.

    Runs the reps=r1 and reps=r2 hardware-loop NEFFs (each dispatch
    executes the full kernel `reps` times back-to-back on-device) and
    returns the wall-clock slope (med2-med1)/(r2-r1), which cancels the
    fixed PJRT/axon dispatch overhead.
    """
    in_maps, npc = _make_in_maps(**inputs)
    apply_gb = not (np.all(np.asarray(inputs["ln_gamma"]) == 1.0)
                    and np.all(np.asarray(inputs["ln_beta"]) == 0.0))
    med1, t1 = _bench_walls(_get_nc(npc, apply_gb, r1), in_maps, iters)
    med2, t2 = _bench_walls(_get_nc(npc, apply_gb, r2), in_maps, iters)
    per_exec = (med2 - med1) / (r2 - r1)
    return per_exec, {"r1": r1, "r2": r2, "med1": med1, "med2": med2,
                      "times1": t1, "times2": t2}


def bench_device(inputs, iters=6):
    """Wall time of a single dispatch (dominated by ~70ms axon/PJRT fixed
    dispatch overhead; kept for reference)."""
    in_maps, npc = _make_in_maps(**inputs)
    apply_gb = not (np.all(np.asarray(inputs["ln_gamma"]) == 1.0)
                    and np.all(np.asarray(inputs["ln_beta"]) == 0.0))
    return _bench_walls(_get_nc(npc, apply_gb, 1), in_maps, iters)


# revision 14
# speedup vs baseline: 297.2233x; 2.0665x over previous
"""Trainium2 Bass kernel for nn_MiddleFusionModule.

out = LayerNorm(node + sigmoid(node@Wg1 + (t@Wg2+bg)[seg]) * t[seg]),
t = relu(text@W1+b1)@W2+b2, over 131072 nodes sharded across 8 cores.

Strategy (one SPMD program, 8 data-parallel cores):
 - Host passes node_feat TRANSPOSED (feature-major [256, N]) so the big
   matmul needs no on-chip input transpose, plus a one-hot segment
   matrix [64, N] so the per-node text gather becomes two tiny-K
   matmuls (uniform across cores).
 - All matmuls run as float32r (full-rate fp32, ~1e-4 rel err).
 - Feature-major epilogue: sigmoid on ACT, gate*t_node on DVE,
   +node on GPSIMD, then PE transpose-mode flips 128x128 blocks to
   node-major PSUM where LayerNorm runs (bn_stats + Newton rsqrt +
   ACT affine).
 - For benchmarking, the same program can be built with reps>1: the
   ENTIRE kernel (weight DMAs + text MLP + main loop) is wrapped in a
   tc.For_i hardware loop, so one NEFF dispatch executes the kernel
   `reps` times back-to-back.  Wall-clock of two dispatches with
   different reps gives the true per-execution HW time as the slope,
   cancelling the fixed PJRT/axon dispatch overhead (~70ms here).
"""

import os
import sys

for _p in ("/opt/trn_rl_repo", "/root/.axon_site/_ro/trn_rl_repo"):
    if os.path.isdir(_p) and _p not in sys.path:
        sys.path.insert(0, _p)

from contextlib import ExitStack

import numpy as np

import concourse.bacc as bacc
import concourse.mybir as mybir
import concourse.tile as tile
from concourse.bass_utils import run_bass_kernel_spmd
from concourse.masks import make_identity

F32 = mybir.dt.float32
F32R = mybir.dt.float32r
AF = mybir.ActivationFunctionType
ET = mybir.EngineType
N_CORES = 8
D = 256          # node dim
TD = 768         # text dim
HD = 1024        # hidden dim
B = 64           # batch (segments)
CHUNK = 512      # nodes per inner chunk
LN_EPS = 1e-3


def _build(npc: int, apply_gb: bool, reps: int = 1, ablate: str = "full"):
    """Build the single SPMD program for `npc` nodes per core.

    reps=1: one kernel execution (the correctness program).
    reps>1: the full kernel body repeats `reps` times inside a hardware
    loop (benchmark program; output identical since each rep is
    idempotent).
    ablate: "empty"|"dma"|"mm"|"epi"|"tr"|"full" — cumulative stage
    subsets for bench-based time attribution (non-"full" programs give
    wrong results; bench only).
    """
    nch = npc // CHUNK
    nc = bacc.Bacc("TRN2", target_bir_lowering=False, debug=False,
                   num_devices=N_CORES)

    nodeT = nc.dram_tensor("nodeT", [D, npc], F32, kind="ExternalInput")
    onehot = nc.dram_tensor("onehot", [B, npc], mybir.dt.uint8, kind="ExternalInput")
    textT = nc.dram_tensor("textT", [TD, B], F32, kind="ExternalInput")
    w1 = nc.dram_tensor("w1", [TD, HD], F32, kind="ExternalInput")
    b1 = nc.dram_tensor("b1", [1, HD], F32, kind="ExternalInput")
    w2 = nc.dram_tensor("w2", [HD, D], F32, kind="ExternalInput")
    b2 = nc.dram_tensor("b2", [1, D], F32, kind="ExternalInput")
    wg1 = nc.dram_tensor("wg1", [D, D], F32, kind="ExternalInput")
    wg2 = nc.dram_tensor("wg2", [D, D], F32, kind="ExternalInput")
    bg = nc.dram_tensor("bg", [1, D], F32, kind="ExternalInput")
    gamma = nc.dram_tensor("gamma", [1, D], F32, kind="ExternalInput")
    beta = nc.dram_tensor("beta", [1, D], F32, kind="ExternalInput")
    onesd = nc.dram_tensor("onesd", [1, B], F32, kind="ExternalInput")
    out = nc.dram_tensor("out", [npc, D], F32, kind="ExternalOutput")

    with tile.TileContext(nc) as tc:
        with ExitStack() as ctx:
            consts = ctx.enter_context(tc.tile_pool(name="consts", bufs=1))

            ident = consts.tile([128, 128], F32)
            make_identity(nc, ident)

            # ---- pools ----
            inp = ctx.enter_context(tc.tile_pool(name="inp", bufs=4))
            work = ctx.enter_context(tc.tile_pool(name="work", bufs=3))
            pz = ctx.enter_context(tc.tile_pool(name="pz", bufs=2, space="PSUM"))
            ptn = ctx.enter_context(tc.tile_pool(name="ptn", bufs=1, space="PSUM"))
            pe_ps = ctx.enter_context(tc.tile_pool(name="pe_ps", bufs=1, space="PSUM"))

            nodeTv = nodeT.bitcast(F32R).rearrange("(c k) n -> k c n", c=2)
            outv = out.rearrange("(ch j p) f -> ch p j f", p=128, j=4)
            outv2 = out.rearrange("(c2 j p) f -> c2 p j f", p=128, j=8)

            def R(ap):
                return ap.bitcast(F32R)

            LV = {"empty": 0, "dma": 1, "mm": 2, "epi": 3, "tr": 4,
                  "full": 5}[ablate]

            def body():
                if LV == 0:
                    dummy = work.tile([128, 8], F32, tag="dummy")
                    nc.vector.memset(dummy, 0.0)
                    return
                # ---- weights / constants into SBUF (every rep) ----
                wg1_sb = consts.tile([128, 2, D], F32R, tag="wg1")
                nc.sync.dma_start(out=wg1_sb, in_=wg1.bitcast(F32R).rearrange("(c k) n -> k c n", c=2))
                b1_sb = consts.tile([1, HD], F32R, tag="b1")
                nc.sync.dma_start(out=b1_sb, in_=b1.bitcast(F32R)[:, :])
                b2_sb = consts.tile([1, D], F32R, tag="b2")
                nc.sync.dma_start(out=b2_sb, in_=b2.bitcast(F32R)[:, :])
                bg_sb = consts.tile([1, D], F32R, tag="bg")
                nc.sync.dma_start(out=bg_sb, in_=bg.bitcast(F32R)[:, :])
                ones64 = consts.tile([1, B], F32R, tag="ones64")
                nc.sync.dma_start(out=ones64, in_=onesd.bitcast(F32R)[:, :])
                t_sb = consts.tile([B, D], F32R, tag="t_sb")    # text rows, node-dim
                u_sb = consts.tile([B, D], F32R, tag="u_sb")    # (t @ Wg2 + bg) rows

                gb_sb = None
                if apply_gb and LV >= 5:
                    gb_sb = consts.tile([128, 2, D], F32, tag="gb")
                    for name, src, slot in (("g", gamma, 0), ("b", beta, 1)):
                        import concourse.bass as bass
                        bcast = bass.AP(tensor=src.ap().tensor, offset=0,
                                        ap=[[0, 128], [1, D]])
                        nc.gpsimd.dma_start(out=gb_sb[:, slot, :], in_=bcast)

                # ---- text MLP (tiny; PSUM borrowed from main-loop pools) ----
                tx_sb = consts.tile([128, 6, B], F32R, tag="tx")
                nc.sync.dma_start(out=tx_sb, in_=textT.bitcast(F32R).rearrange("(c k) m -> k c m", c=6))
                w1_sb = consts.tile([128, 6, HD], F32R, tag="w1")
                nc.sync.dma_start(out=w1_sb, in_=w1.bitcast(F32R).rearrange("(c k) n -> k c n", c=6))
                w2_sb = consts.tile([128, 8, D], F32R, tag="w2")
                nc.sync.dma_start(out=w2_sb, in_=w2.bitcast(F32R).rearrange("(c k) n -> k c n", c=8))
                wg2_sb = consts.tile([128, 2, D], F32R, tag="wg2")
                nc.sync.dma_start(out=wg2_sb, in_=wg2.bitcast(F32R).rearrange("(c k) n -> k c n", c=2))

                def mlp():
                    ps_t1 = pz.tile([128, 2, CHUNK], F32, tag="ps_z")
                    for h in range(2):
                        for k in range(6):
                            nc.tensor.matmul(
                                ps_t1[:B, h, :], R(tx_sb[:, k, :]),
                                R(w1_sb[:, k, h * 512:(h + 1) * 512]),
                                start=(k == 0), stop=False)
                        nc.tensor.matmul(
                            ps_t1[:B, h, :], R(ones64),
                            R(b1_sb[:, h * 512:(h + 1) * 512]),
                            start=False, stop=True)
                    t1_sb = consts.tile([B, 2, 512], F32, tag="t1")
                    for h in range(2):
                        nc.scalar.activation(out=t1_sb[:, h, :], in_=ps_t1[:B, h, :],
                                             func=AF.Relu)
                    # transpose t1 -> t1T [1024, 64] as [128, 8, 64]
                    t1T_sb = consts.tile([128, 8, B], F32R, tag="t1T")
                    ps_tr = pe_ps.tile([128, 2, CHUNK], F32, tag="ps_e")
                    for j in range(8):
                        src = t1_sb[:, j // 4, (j % 4) * 128:(j % 4 + 1) * 128]
                        nc.tensor.matmul(ps_tr[:, 0, :B], src, ident[:B, :B],
                                         is_transpose=True, start=True, stop=True)
                        nc.vector.tensor_copy(out=t1T_sb[:, j, :], in_=ps_tr[:, 0, :B])
                    ps_t = ptn.tile([128, 2, CHUNK], F32, tag="ps_tn")
                    for j in range(8):
                        nc.tensor.matmul(ps_t[:B, 0, :D], R(t1T_sb[:, j, :]), R(w2_sb[:, j, :]),
                                         start=(j == 0), stop=False)
                    nc.tensor.matmul(ps_t[:B, 0, :D], R(ones64), R(b2_sb), start=False, stop=True)
                    nc.vector.tensor_copy(out=t_sb, in_=ps_t[:B, 0, :D])
                    # transpose t -> tT [256, 64] as [128, 2, 64]
                    tT_sb = consts.tile([128, 2, B], F32R, tag="tT")
                    for c in range(2):
                        nc.tensor.matmul(ps_tr[:, 0, :B], t_sb[:, c * 128:(c + 1) * 128].bitcast(F32),
                                         ident[:B, :B],
                                         is_transpose=True, start=True, stop=True)
                        nc.vector.tensor_copy(out=tT_sb[:, c, :], in_=ps_tr[:, 0, :B])
                    ps_u = ptn.tile([128, 2, CHUNK], F32, tag="ps_tn")
                    for c in range(2):
                        nc.tensor.matmul(ps_u[:B, 0, :D], R(tT_sb[:, c, :]), R(wg2_sb[:, c, :]),
                                         start=(c == 0), stop=False)
                    nc.tensor.matmul(ps_u[:B, 0, :D], R(ones64), R(bg_sb), start=False, stop=True)
                    nc.vector.tensor_copy(out=u_sb, in_=ps_u[:B, 0, :D])

                if LV >= 2:
                    mlp()

                # ---- main loop ----
                dma_cache = {}

                def front_half(ch):
                    """DMA-in + matmuls + sigmoid/mul/add for chunk ch.
                    Returns the live enh tile for the back half."""
                    # node: 2-chunk DMAs on the SP ring; onehot: 4-chunk DMAs
                    # via SWDGE (gpsimd) so the two never share a DGE queue.
                    if ch % 2 == 0:
                        n2 = inp.tile([128, 2, 2 * CHUNK], F32R, tag="node2")
                        hi = min((ch + 2) * CHUNK, npc)
                        nc.sync.dma_start(out=n2[:, :, :hi - ch * CHUNK],
                                          in_=nodeTv[:, :, ch * CHUNK:hi])
                        dma_cache["node"] = n2
                    if ch % 4 == 0:
                        o4 = inp.tile([B, 4 * CHUNK], F32R, tag="oh4")
                        hi = min((ch + 4) * CHUNK, npc)
                        # SWDGE casts uint8 -> f32r during the transfer, so the
                        # one-hot matrix costs 1 byte/elem of HBM instead of 4
                        nc.gpsimd.dma_start(out=o4[:, :hi - ch * CHUNK],
                                            in_=onehot[:, ch * CHUNK:hi])
                        dma_cache["oh"] = o4
                    node_sb = dma_cache["node"][:, :, (ch % 2) * CHUNK:
                                                (ch % 2 + 1) * CHUNK]
                    oh_sb = dma_cache["oh"][:, (ch % 4) * CHUNK:(ch % 4 + 1) * CHUNK]

                    if LV < 2:
                        return None
                    ps_z = pz.tile([128, 2, CHUNK], F32, tag="ps_z")
                    ps_tn = ptn.tile([128, 2, CHUNK], F32, tag="ps_tn")
                    for c in range(2):
                        for k in range(2):
                            nc.tensor.matmul(
                                ps_z[:, c, :],
                                R(wg1_sb[:, k, c * 128:(c + 1) * 128]),
                                R(node_sb[:, k, :]),
                                start=(k == 0), stop=False)
                        nc.tensor.matmul(
                            ps_z[:, c, :], R(u_sb[:, c * 128:(c + 1) * 128]),
                            R(oh_sb), start=False, stop=True)
                        nc.tensor.matmul(
                            ps_tn[:, c, :], R(t_sb[:, c * 128:(c + 1) * 128]),
                            R(oh_sb), start=True, stop=True)

                    if LV < 3:
                        return None
                    gate_sb = work.tile([128, 2, CHUNK], F32, tag="gate")
                    gt_sb = work.tile([128, 2, CHUNK], F32, tag="gt")
                    enh_sb = work.tile([128, 2, CHUNK], F32, tag="enh")
                    # sigmoid/mul stay per-PSUM-bank (ops must not cross a
                    # bank); the SBUF-only add fuses both banks into one
                    # GPSIMD op to amortize its dispatch cost.
                    for c in range(2):
                        nc.scalar.activation(out=gate_sb[:, c, :],
                                             in_=ps_z[:, c, :], func=AF.Sigmoid)
                        nc.vector.tensor_mul(out=gt_sb[:, c, :],
                                             in0=gate_sb[:, c, :],
                                             in1=ps_tn[:, c, :])
                    nc.gpsimd.tensor_add(out=enh_sb[:, :, :],
                                         in0=gt_sb[:, :, :],
                                         in1=node_sb[:, :, :].bitcast(F32))
                    return enh_sb

                def back_half(ch, enh_sb):
                    """Transpose + LayerNorm + store for chunk ch."""
                    if LV >= 4 and enh_sb is not None:
                        ps_e = pe_ps.tile([128, 2, CHUNK], F32, tag="ps_e")
                        for j in range(4):
                            for c in range(2):
                                nc.tensor.matmul(
                                    ps_e[:, j // 2, (j % 2) * 256 + c * 128:
                                         (j % 2) * 256 + (c + 1) * 128],
                                    enh_sb[:, c, j * 128:(j + 1) * 128],
                                    ident, is_transpose=True,
                                    start=True, stop=True, skip_group_check=True)
                    if LV < 5:
                        if ch % 2 == 0:
                            out2_sb = work.tile([128, 8, D], F32, tag="out2")
                            nc.gpsimd.memset(out2_sb, 0.0)
                            dma_cache["out2"] = out2_sb
                        if ch % 2 == 1:
                            nc.scalar.dma_start(out=outv2[ch // 2],
                                                in_=dma_cache["out2"])
                        elif ch == nch - 1:
                            nc.scalar.dma_start(out=outv[ch],
                                                in_=dma_cache["out2"][:, 0:4, :])
                        return

                    st_sb = work.tile([128, 2, 2, 6], F32, tag="st")
                    mv_sb = work.tile([128, 2, 2, 2], F32, tag="mv")
                    for b in range(2):
                        for g in range(2):
                            nc.vector.bn_stats(
                                out=st_sb[:, b, g, :],
                                in_=ps_e[:, b, g * 256:(g + 1) * 256])
                            nc.vector.bn_aggr(out=mv_sb[:, b, g, :],
                                              in_=st_sb[:, b, g:g + 1, :])
                    # rstd = 1/sqrt(var+eps): recip-seeded Newton (1 iter; var~1.1)
                    ve = work.tile([128, 2, 2, 1], F32, tag="ve")
                    y = work.tile([128, 2, 2, 1], F32, tag="y")
                    tmp = work.tile([128, 2, 2, 1], F32, tag="tmp")
                    negms = work.tile([128, 2, 2, 1], F32, tag="negms")
                    nc.vector.tensor_scalar_add(out=ve, in0=mv_sb[:, :, :, 1:2],
                                                scalar1=LN_EPS)
                    nc.vector.reciprocal(out=y, in_=ve)
                    nc.vector.tensor_scalar(out=y, in0=y, scalar1=0.5, scalar2=0.5,
                                            op0=mybir.AluOpType.mult,
                                            op1=mybir.AluOpType.add)
                    for _ in range(2):
                        nc.vector.tensor_mul(out=tmp, in0=y, in1=y)
                        nc.vector.tensor_mul(out=tmp, in0=tmp, in1=ve)
                        nc.vector.tensor_scalar(out=tmp, in0=tmp, scalar1=-0.5,
                                                scalar2=1.5,
                                                op0=mybir.AluOpType.mult,
                                                op1=mybir.AluOpType.add)
                        nc.vector.tensor_mul(out=y, in0=y, in1=tmp)
                    nc.vector.tensor_mul(out=negms, in0=mv_sb[:, :, :, 0:1], in1=y)
                    nc.vector.tensor_scalar_mul(out=negms, in0=negms, scalar1=-1.0)

                    # pair output tiles of two chunks into one 1MB DMA
                    if ch % 2 == 0:
                        out2_sb = work.tile([128, 8, D], F32, tag="out2")
                        dma_cache["out2"] = out2_sb
                    out_sb = dma_cache["out2"][:, (ch % 2) * 4:(ch % 2) * 4 + 4, :]
                    for b in range(2):
                        for g in range(2):
                            j = 2 * b + g
                            nc.scalar.activation(
                                out=out_sb[:, j, :],
                                in_=ps_e[:, b, g * 256:(g + 1) * 256],
                                func=AF.Identity,
                                bias=negms[:, b, g, :], scale=y[:, b, g, :])
                    if apply_gb:
                        for j in range(4):
                            nc.vector.tensor_mul(out=out_sb[:, j, :],
                                                 in0=out_sb[:, j, :],
                                                 in1=gb_sb[:, 0, :])
                            nc.vector.tensor_add(out=out_sb[:, j, :],
                                                 in0=out_sb[:, j, :],
                                                 in1=gb_sb[:, 1, :])
                    if ch % 2 == 1:
                        nc.scalar.dma_start(out=outv2[ch // 2],
                                            in_=dma_cache["out2"])
                    elif ch == nch - 1:
                        nc.scalar.dma_start(out=outv[ch],
                                            in_=dma_cache["out2"][:, 0:4, :])

                # one-chunk software pipeline: chunk i's front half is emitted
                # before chunk i-1's back half so PE/ACT/DVE streams always have
                # ready work ahead of the cross-engine dependency chain.
                prev_enh = None
                for ch in range(nch + 1):
                    if ch < nch:
                        cur_enh = front_half(ch)
                    else:
                        cur_enh = None
                    if ch >= 1:
                        back_half(ch - 1, prev_enh)
                    prev_enh = cur_enh

            if reps == 1:
                body()
            else:
                with tc.For_i(0, reps, 1,
                              hint_engines=(ET.PE, ET.DVE, ET.Activation,
                                            ET.Pool, ET.SP)):
                    body()

    nc.compile()
    return nc


_NC_CACHE = {}


def _get_nc(npc, apply_gb, reps=1):
    key = (npc, apply_gb, reps)
    if key not in _NC_CACHE:
        _NC_CACHE[key] = _build(npc, apply_gb, reps)
    return _NC_CACHE[key]


def _make_in_maps(node_feat, text_feat, segment_ids, W1, b1, W2, b2, Wg, bg,
                  ln_gamma, ln_beta):
    total = node_feat.shape[0]
    npc = total // N_CORES
    node_feat = np.asarray(node_feat, dtype=np.float32)
    nodeT = np.ascontiguousarray(node_feat.T)               # [256, total]
    seg = np.asarray(segment_ids)
    onehot = (seg[None, :] == np.arange(B, dtype=seg.dtype)[:, None]
              ).astype(np.uint8)                            # [64, total]
    shared = {
        "textT": np.ascontiguousarray(np.asarray(text_feat, np.float32).T),
        "w1": np.asarray(W1, np.float32),
        "b1": np.asarray(b1, np.float32).reshape(1, HD),
        "w2": np.asarray(W2, np.float32),
        "b2": np.asarray(b2, np.float32).reshape(1, D),
        "wg1": np.ascontiguousarray(np.asarray(Wg, np.float32)[:D]),
        "wg2": np.ascontiguousarray(np.asarray(Wg, np.float32)[D:]),
        "bg": np.asarray(bg, np.float32).reshape(1, D),
        "gamma": np.asarray(ln_gamma, np.float32).reshape(1, D),
        "beta": np.asarray(ln_beta, np.float32).reshape(1, D),
        "onesd": np.ones((1, B), np.float32),
    }
    in_maps = []
    for c in range(N_CORES):
        m = dict(shared)
        m["nodeT"] = np.ascontiguousarray(nodeT[:, c * npc:(c + 1) * npc])
        m["onehot"] = np.ascontiguousarray(onehot[:, c * npc:(c + 1) * npc])
        in_maps.append(m)
    return in_maps, npc


def kernel(node_feat, text_feat, segment_ids, W1, b1, W2, b2, Wg, bg,
           ln_gamma, ln_beta):
    total, d = node_feat.shape
    npc = total // N_CORES
    assert npc % CHUNK == 0

    apply_gb = not (np.all(np.asarray(ln_gamma) == 1.0)
                    and np.all(np.asarray(ln_beta) == 0.0))
    in_maps, npc = _make_in_maps(node_feat, text_feat, segment_ids, W1, b1,
                                 W2, b2, Wg, bg, ln_gamma, ln_beta)
    nc = _get_nc(npc, apply_gb, 1)
    res = run_bass_kernel_spmd(nc, in_maps, core_ids=list(range(N_CORES)))
    out = np.concatenate([res.results[c]["out"] for c in range(N_CORES)], axis=0)
    return out.astype(np.float32)


# ---------------------------------------------------------------------------
# benchmarking helpers (used by test.py; not needed for grading correctness)
# ---------------------------------------------------------------------------

def _make_sharded(nc):
    import jax
    from jax.experimental.shard_map import shard_map
    from jax.sharding import Mesh, PartitionSpec
    import concourse.bass2jax as b2j
    import concourse.mybir as mb

    b2j.install_neuronx_cc_hook()
    partition_name = (nc.partition_id_tensor.name
                      if nc.partition_id_tensor else None)
    in_names, out_names, out_avals = [], [], []
    for alloc in nc.m.functions[0].allocations:
        if not isinstance(alloc, mb.MemoryLocationSet):
            continue
        name = alloc.memorylocations[0].name
        if alloc.kind == "ExternalInput":
            if name != partition_name:
                in_names.append(name)
        elif alloc.kind == "ExternalOutput":
            out_names.append(name)
            out_avals.append(jax.core.ShapedArray(tuple(alloc.tensor_shape),
                                                  mb.dt.np(alloc.dtype)))
    n_params = len(in_names)
    n_outs = len(out_avals)
    in_names_all = list(in_names) + out_names
    if partition_name is not None:
        in_names_all.append(partition_name)
    donate = tuple(range(n_params, n_params + n_outs))

    def _body(*args):
        operands = list(args)
        if partition_name is not None:
            operands.append(b2j.partition_id_tensor())
        outs = b2j._bass_exec_p.bind(
            *operands, out_avals=tuple(out_avals), in_names=tuple(in_names_all),
            out_names=tuple(out_names), lowering_input_output_aliases=(),
            sim_require_finite=True, sim_require_nnan=True, nc=nc)
        return tuple(outs)

    devices = jax.devices()[:N_CORES]
    mesh = Mesh(np.asarray(devices), ("core",))
    sharded = jax.jit(
        shard_map(_body, mesh=mesh,
                  in_specs=(PartitionSpec("core"),) * (n_params + n_outs),
                  out_specs=(PartitionSpec("core"),) * n_outs,
                  check_rep=False),
        donate_argnums=donate, keep_unused=True)
    return sharded, in_names, out_avals


def _bench_walls(nc, in_maps, iters):
    """Median + samples of wall time for one dispatch of `nc` (device-resident
    inputs, donated output scratch reused across iterations)."""
    import time
    import jax
    from jax.sharding import Mesh, PartitionSpec

    sharded, in_names, out_avals = _make_sharded(nc)
    mesh = Mesh(np.asarray(jax.devices()[:N_CORES]), ("core",))
    sh = jax.sharding.NamedSharding(mesh, PartitionSpec("core"))
    concat_in = [
        np.concatenate([np.asarray(in_maps[c][nm]) for c in range(N_CORES)],
                       axis=0)
        for nm in in_names]
    in_dev = [jax.device_put(a, sh) for a in concat_in]
    outs = [jax.device_put(
        np.zeros((N_CORES * z.shape[0], *z.shape[1:]), z.dtype), sh)
        for z in out_avals]
    jax.block_until_ready(in_dev + outs)
    # warmup (compile + first exec)
    outs = sharded(*in_dev, *outs)
    jax.block_until_ready(outs)
    times = []
    for _ in range(iters):
        t0 = time.perf_counter()
        outs = sharded(*in_dev, *outs)
        jax.block_until_ready(outs)
        times.append(time.perf_counter() - t0)
    times.sort()
    return times[len(times) // 2], times


def bench_hw(inputs, r1=16, r2=1040, iters=11):
    """True per-execution HW time of the kernel.

    Runs the reps=r1 and reps=r2 hardware-loop NEFFs (each dispatch
    executes the full kernel `reps` times back-to-back on-device) and
    returns the wall-clock slope (med2-med1)/(r2-r1), which cancels the
    fixed PJRT/axon dispatch overhead.
    """
    in_maps, npc = _make_in_maps(**inputs)
    apply_gb = not (np.all(np.asarray(inputs["ln_gamma"]) == 1.0)
                    and np.all(np.asarray(inputs["ln_beta"]) == 0.0))
    med1, t1 = _bench_walls(_get_nc(npc, apply_gb, r1), in_maps, iters)
    med2, t2 = _bench_walls(_get_nc(npc, apply_gb, r2), in_maps, iters)
    per_exec = (med2 - med1) / (r2 - r1)
    return per_exec, {"r1": r1, "r2": r2, "med1": med1, "med2": med2,
                      "times1": t1, "times2": t2}


def bench_device(inputs, iters=6):
    """Wall time of a single dispatch (dominated by ~70ms axon/PJRT fixed
    dispatch overhead; kept for reference)."""
    in_maps, npc = _make_in_maps(**inputs)
    apply_gb = not (np.all(np.asarray(inputs["ln_gamma"]) == 1.0)
                    and np.all(np.asarray(inputs["ln_beta"]) == 0.0))
    return _bench_walls(_get_nc(npc, apply_gb, 1), in_maps, iters)


# revision 15
# speedup vs baseline: 297.7399x; 1.0017x over previous
"""Trainium2 Bass kernel for nn_MiddleFusionModule — v2 (bf16 streams).

out = LayerNorm(node + sigmoid(node@Wg1 + (t@Wg2+bg)[seg]) * t[seg]),
t = relu(text@W1+b1)@W2+b2, over 131072 nodes sharded across 8 cores.

v2 design (one SPMD program, 8 data-parallel cores):
 - All heavy streams in bf16: node input (feature-major [256,N]),
   weights, one-hot matrix (uint8 in HBM, SWDGE-cast to bf16), and the
   output (host converts back to f32).  Halves HBM traffic vs f32.
 - z = Wg1.T-slices @ node (K=2x128) + u.T @ oh (K=64) accumulated in
   PSUM; tn = t.T @ oh.  The two K=64 matmuls are row-packed into
   disjoint PE row-groups (u rows 0-63, t rows 64-127) so they run
   concurrently; the one-hot tile is partition-replicated for this.
 - Epilogue: sigmoid on ACT -> gate(bf16), gate*tn on DVE -> gt(bf16).
   The "+ node" add happens FOR FREE in PSUM: the PE transpose of gt
   and the PE transpose of node accumulate into the same PSUM bank
   (matmul start/stop), yielding node-major enhanced directly.
 - LayerNorm on node-major PSUM: bn_stats/bn_aggr (DVE),
   rstd = sqrt(reciprocal_approx_fast(var+eps)), affine on ACT with
   per-partition scale/bias, bf16 output, grouped 0.5MB output DMAs.
 - reps>1 wraps the whole body (incl. weight DMAs + text MLP) in a
   tc.For_i hardware loop for slope-based wall-clock benchmarking.
"""

import os
import sys

for _p in ("/opt/trn_rl_repo", "/root/.axon_site/_ro/trn_rl_repo"):
    if os.path.isdir(_p) and _p not in sys.path:
        sys.path.insert(0, _p)

from contextlib import ExitStack

import numpy as np

import concourse.bacc as bacc
import concourse.mybir as mybir
import concourse.tile as tile
from concourse.bass_utils import run_bass_kernel_spmd
from concourse.masks import make_identity

F32 = mybir.dt.float32
BF16 = mybir.dt.bfloat16
AF = mybir.ActivationFunctionType
ET = mybir.EngineType
N_CORES = 8
D = 256          # node dim
TD = 768         # text dim
HD = 1024        # hidden dim
B = 64           # batch (segments)
CHUNK = 512      # nodes per inner chunk
LN_EPS = 1e-3


def _build(npc: int, apply_gb: bool, reps: int = 1, ablate: str = "full",
           opts: dict | None = None):
    LV = {"empty": 0, "dma": 1, "mm": 2, "sig": 3, "add": 4, "tr": 5,
          "bn": 6, "full": 7}[ablate]
    O = {"skew": 1, "add_engine": "gpsimd", "work_bufs": 3, "inp_bufs": 2,
         "psum": (2, 2, 4), "bf16_pe": True, "newton_iters": 2,
         "ilv_stats": False}
    O.update(opts or {})
    nch = npc // CHUNK
    assert nch % 4 == 0
    nc = bacc.Bacc("TRN2", target_bir_lowering=False, debug=False,
                   num_devices=N_CORES)

    nodeT = nc.dram_tensor("nodeT", [D, npc], BF16, kind="ExternalInput")
    onehot = nc.dram_tensor("onehot", [B, npc], BF16, kind="ExternalInput")
    textT = nc.dram_tensor("textT", [TD, B], BF16, kind="ExternalInput")
    w1 = nc.dram_tensor("w1", [TD, HD], BF16, kind="ExternalInput")
    b1 = nc.dram_tensor("b1", [1, HD], BF16, kind="ExternalInput")
    w2 = nc.dram_tensor("w2", [HD, D], BF16, kind="ExternalInput")
    b2 = nc.dram_tensor("b2", [1, D], BF16, kind="ExternalInput")
    wg1 = nc.dram_tensor("wg1", [D, D], BF16, kind="ExternalInput")
    wg2 = nc.dram_tensor("wg2", [D, D], BF16, kind="ExternalInput")
    bg = nc.dram_tensor("bg", [1, D], BF16, kind="ExternalInput")
    gamma = nc.dram_tensor("gamma", [1, D], F32, kind="ExternalInput")
    beta = nc.dram_tensor("beta", [1, D], F32, kind="ExternalInput")
    onesd = nc.dram_tensor("onesd", [1, B], BF16, kind="ExternalInput")
    out = nc.dram_tensor("out", [npc, D], BF16, kind="ExternalOutput")

    with tile.TileContext(nc) as tc:
        with ExitStack() as ctx:
            consts = ctx.enter_context(tc.tile_pool(name="consts", bufs=1))

            ident32 = consts.tile([128, 128], F32)
            make_identity(nc, ident32)
            identb = consts.tile([128, 128], BF16)
            make_identity(nc, identb)

            inp = ctx.enter_context(tc.tile_pool(name="inp", bufs=O["inp_bufs"]))
            work = ctx.enter_context(tc.tile_pool(name="work", bufs=O["work_bufs"]))
            pzc = ctx.enter_context(tc.tile_pool(name="pzc", bufs=O["psum"][0], space="PSUM"))
            ptnc = ctx.enter_context(tc.tile_pool(name="ptnc", bufs=O["psum"][1], space="PSUM"))
            pec = ctx.enter_context(tc.tile_pool(name="pec", bufs=O["psum"][2], space="PSUM"))

            nodeTv = nodeT.rearrange("(c k) n -> k c n", c=2)
            # out viewed for grouped stores: 4 chunks -> [128, 16, 256]
            outv4 = out.rearrange("(c4 j p) f -> c4 p j f", p=128, j=16)

            def body():
                if LV == 0:
                    dummy = work.tile([128, 8], F32, tag="dummy")
                    nc.vector.memset(dummy, 0.0)
                    return
                # ---- weights / constants (bf16) ----
                wg1_sb = consts.tile([128, 2, D], BF16, tag="wg1")
                nc.sync.dma_start(out=wg1_sb, in_=wg1.rearrange("(c k) n -> k c n", c=2))
                b1_sb = consts.tile([1, HD], BF16, tag="b1")
                nc.sync.dma_start(out=b1_sb, in_=b1[:, :])
                b2_sb = consts.tile([1, D], BF16, tag="b2")
                nc.sync.dma_start(out=b2_sb, in_=b2[:, :])
                bg_sb = consts.tile([1, D], BF16, tag="bg")
                nc.sync.dma_start(out=bg_sb, in_=bg[:, :])
                ones64 = consts.tile([1, B], BF16, tag="ones64")
                nc.sync.dma_start(out=ones64, in_=onesd[:, :])
                # u rows 0-63, t rows 64-127 (row-packed K=64 stationaries)
                ut_sb = consts.tile([128, D], BF16, tag="ut_sb")

                gb_sb = None
                if apply_gb:
                    gb_sb = consts.tile([128, 2, D], F32, tag="gb")
                    import concourse.bass as bass
                    for src, slot in ((gamma, 0), (beta, 1)):
                        bcast = bass.AP(tensor=src.ap().tensor, offset=0,
                                        ap=[[0, 128], [1, D]])
                        nc.gpsimd.dma_start(out=gb_sb[:, slot, :], in_=bcast)

                # ---- text MLP (bf16 operands, f32 PSUM) ----
                tx_sb = consts.tile([128, 6, B], BF16, tag="tx")
                nc.sync.dma_start(out=tx_sb, in_=textT.rearrange("(c k) m -> k c m", c=6))
                w1_sb = consts.tile([128, 6, HD], BF16, tag="w1")
                nc.sync.dma_start(out=w1_sb, in_=w1.rearrange("(c k) n -> k c n", c=6))
                w2_sb = consts.tile([128, 8, D], BF16, tag="w2")
                nc.sync.dma_start(out=w2_sb, in_=w2.rearrange("(c k) n -> k c n", c=8))
                wg2_sb = consts.tile([128, 2, D], BF16, tag="wg2")
                nc.sync.dma_start(out=wg2_sb, in_=wg2.rearrange("(c k) n -> k c n", c=2))

                def mlp():
                    ps_h0 = pzc.tile([128, CHUNK], F32, tag="ps_z")
                    ps_h1 = pzc.tile([128, CHUNK], F32, tag="ps_z")
                    t1_sb = consts.tile([B, 2, 512], F32, tag="t1")
                    for h, ps in ((0, ps_h0), (1, ps_h1)):
                        for k in range(6):
                            nc.tensor.matmul(
                                ps[:B, :], tx_sb[:, k, :],
                                w1_sb[:, k, h * 512:(h + 1) * 512],
                                start=(k == 0), stop=False)
                        nc.tensor.matmul(
                            ps[:B, :], ones64, b1_sb[:, h * 512:(h + 1) * 512],
                            start=False, stop=True)
                        nc.scalar.activation(out=t1_sb[:, h, :], in_=ps[:B, :],
                                             func=AF.Relu)
                    # transpose t1 -> t1T [1024, 64] as [128, 8, 64]
                    t1T_sb = consts.tile([128, 8, B], BF16, tag="t1T")
                    for j in range(8):
                        ps_tr = pec.tile([128, 2, D], F32, tag="ps_e")
                        src = t1_sb[:, j // 4, (j % 4) * 128:(j % 4 + 1) * 128]
                        nc.tensor.matmul(ps_tr[:, 0, :B], src, ident32[:B, :B],
                                         is_transpose=True, start=True, stop=True)
                        nc.vector.tensor_copy(out=t1T_sb[:, j, :], in_=ps_tr[:, 0, :B])
                    ps_t = ptnc.tile([128, CHUNK], F32, tag="ps_tn")
                    for j in range(8):
                        nc.tensor.matmul(ps_t[:B, :D], t1T_sb[:, j, :], w2_sb[:, j, :],
                                         start=(j == 0), stop=False)
                    nc.tensor.matmul(ps_t[:B, :D], ones64, b2_sb, start=False, stop=True)
                    nc.vector.tensor_copy(out=ut_sb[B:, :], in_=ps_t[:B, :D])
                    # transpose t -> tT [256, 64] as [128, 2, 64]
                    tT_sb = consts.tile([128, 2, B], BF16, tag="tT")
                    t_f32 = consts.tile([B, D], F32, tag="t_f32")
                    nc.vector.tensor_copy(out=t_f32, in_=ps_t[:B, :D])
                    for c in range(2):
                        ps_tr2 = pec.tile([128, 2, D], F32, tag="ps_e")
                        nc.tensor.matmul(ps_tr2[:, 0, :B], t_f32[:, c * 128:(c + 1) * 128],
                                         ident32[:B, :B],
                                         is_transpose=True, start=True, stop=True)
                        nc.vector.tensor_copy(out=tT_sb[:, c, :], in_=ps_tr2[:, 0, :B])
                    ps_u = ptnc.tile([128, CHUNK], F32, tag="ps_tn")
                    for c in range(2):
                        nc.tensor.matmul(ps_u[:B, :D], tT_sb[:, c, :], wg2_sb[:, c, :],
                                         start=(c == 0), stop=False)
                    nc.tensor.matmul(ps_u[:B, :D], ones64, bg_sb, start=False, stop=True)
                    nc.vector.tensor_copy(out=ut_sb[:B, :], in_=ps_u[:B, :D])

                if LV >= 2:
                    mlp()

                # ---- main loop ----
                import concourse.bass as bass
                dma_cache = {}

                def front_half(ch):
                    """DMA-in + matmuls + sigmoid/mul for chunk ch.
                    Returns (gt0, gt1, node_view) for the back half."""
                    if ch % 4 == 0:
                        hi = min((ch + 4) * CHUNK, npc)
                        n4 = inp.tile([128, 2, 4 * CHUNK], BF16, tag="node4")
                        nc.sync.dma_start(out=n4[:, :, :hi - ch * CHUNK],
                                          in_=nodeTv[:, :, ch * CHUNK:hi])
                        dma_cache["node"] = n4
                        # one-hot, partition-replicated x2 (rows 0-63 ==
                        # rows 64-127) for the row-packed K=64 matmuls;
                        # SWDGE casts uint8 -> bf16 during the transfer.
                        o4 = inp.tile([128, 4 * CHUNK], BF16, tag="oh4")
                        nc.sync.dma_start(out=o4[:B, :hi - ch * CHUNK],
                                          in_=onehot[:, ch * CHUNK:hi])
                        nc.sync.dma_start(out=o4[B:, :hi - ch * CHUNK],
                                          in_=onehot[:, ch * CHUNK:hi])
                        dma_cache["oh"] = o4
                    node_sb = dma_cache["node"][:, :, (ch % 4) * CHUNK:
                                                (ch % 4 + 1) * CHUNK]
                    oh_sb = dma_cache["oh"][:, (ch % 4) * CHUNK:(ch % 4 + 1) * CHUNK]

                    if LV < 2:
                        return None
                    gts = []
                    for c in range(2):
                        ps_z = pzc.tile([128, CHUNK], F32, tag="ps_z")
                        ps_tn = ptnc.tile([128, CHUNK], F32, tag="ps_tn")
                        for k in range(2):
                            nc.tensor.matmul(
                                ps_z, wg1_sb[:, k, c * 128:(c + 1) * 128],
                                node_sb[:, k, :], start=(k == 0), stop=False)
                        nc.tensor.matmul(
                            ps_z, ut_sb[:B, c * 128:(c + 1) * 128],
                            oh_sb[:B, :], start=False, stop=True)
                        nc.tensor.matmul(
                            ps_tn, ut_sb[B:, c * 128:(c + 1) * 128],
                            oh_sb[B:, :], start=True, stop=True)
                        if LV < 3:
                            continue
                        gate_sb = work.tile([128, CHUNK], BF16, tag=f"gate{c}")
                        nc.scalar.activation(out=gate_sb, in_=ps_z,
                                             func=AF.Sigmoid)
                        if c == 0:
                            gt2 = work.tile([128, 2, CHUNK], BF16, tag="gt2")
                            gts.append(gt2)
                        nc.vector.tensor_mul(out=gts[0][:, c, :], in0=gate_sb,
                                             in1=ps_tn)
                    if LV < 4:
                        return None
                    # enh = gt + node, one fused add (f32 enh unless the
                    # bf16-PSUM transpose path is enabled)
                    enh_dt = BF16 if O["bf16_pe"] else F32
                    enh_sb = work.tile([128, 2, CHUNK], enh_dt, tag="enh")
                    if O["add_engine"] == "gpsimd":
                        nc.gpsimd.tensor_add(out=enh_sb, in0=gts[0], in1=node_sb)
                    elif O["add_engine"] == "vector":
                        nc.vector.tensor_add(out=enh_sb, in0=gts[0], in1=node_sb)
                    else:
                        nc.gpsimd.tensor_add(out=enh_sb[:, 0, :],
                                             in0=gts[0][:, 0, :],
                                             in1=node_sb[:, 0, :])
                        nc.vector.tensor_add(out=enh_sb[:, 1, :],
                                             in0=gts[0][:, 1, :],
                                             in1=node_sb[:, 1, :])
                    return (enh_sb,)

                def backA(ch, enh_sb):
                    """PE transpose + bn stats + rstd chain for chunk ch."""
                    if LV < 5 or enh_sb is None:
                        return (None, None, None)
                    pes = []
                    st_sb = work.tile([128, 4, 6], F32, tag="st")
                    mv_sb = None
                    if not O["ilv_stats"]:
                        mv_sb = work.tile([128, 4, 2], F32, tag="mv")
                    pe_dt = BF16 if O["bf16_pe"] else F32
                    tr_ident = identb if O["bf16_pe"] else ident32
                    ilv = O["ilv_stats"]
                    for g2 in range(2):           # two node-group pairs
                        if ilv:
                            # groups interleaved along free: idx = 2f + jj.
                            # One FD=512 bn_stats then yields group 0 as its
                            # "even" stats and group 1 as its "odd" stats —
                            # no bn_aggr needed.
                            pe = pec.tile([128, D, 2], pe_dt, tag="ps_e")
                        else:
                            pe = pec.tile([128, 2, D], pe_dt, tag="ps_e")
                        pes.append(pe)
                        for jj in range(2):
                            j = g2 * 2 + jj
                            for c in range(2):
                                dst = (pe[:, c * 128:(c + 1) * 128, jj]
                                       if ilv else
                                       pe[:, jj, c * 128:(c + 1) * 128])
                                nc.tensor.matmul(
                                    dst,
                                    enh_sb[:, c, j * 128:(j + 1) * 128],
                                    tr_ident, is_transpose=True,
                                    start=True, stop=True,
                                    skip_group_check=True)
                        if LV < 6:
                            continue
                        if ilv:
                            nc.vector.bn_stats(
                                out=st_sb[:, g2 * 2, :],
                                in_=pe[:, :, :].rearrange("p a b -> p (a b)"))
                        else:
                            for jj in range(2):
                                g = g2 * 2 + jj
                                nc.vector.bn_stats(out=st_sb[:, g, :],
                                                   in_=pe[:, jj, :])
                                nc.vector.bn_aggr(out=mv_sb[:, g, :],
                                                  in_=st_sb[:, g:g + 1, :])
                    if LV < 6:
                        return (None, None, None)
                    # rstd = 1/sqrt(var+eps): recip-seeded Newton, batched
                    # per chunk (keeps ACT on the sigmoid table set — a
                    # table-set switch costs ~2.7us)
                    ve = work.tile([128, 2, 2], F32, tag="ve")
                    y = work.tile([128, 2, 2], F32, tag="y")
                    tmp = work.tile([128, 2, 2], F32, tag="tmp")
                    negms = work.tile([128, 2, 2], F32, tag="negms")
                    if ilv:
                        # st slot 2*g2 fields: [cnt_e, mu_e, M2_e,
                        # cnt_o, mu_o, M2_o]; var = M2/256
                        st5 = st_sb[:, :, :].rearrange(
                            "p (a b) (c d) -> p a b c d", b=2, d=3)
                        var_view = st5[:, :, 0, :, 2]
                        mu_view = st5[:, :, 0, :, 1]
                        nc.vector.tensor_scalar(out=ve, in0=var_view,
                                                scalar1=1.0 / D,
                                                scalar2=LN_EPS,
                                                op0=mybir.AluOpType.mult,
                                                op1=mybir.AluOpType.add)
                    else:
                        mu_view = mv_sb[:, :, 0:1].rearrange("p (a b) c -> p a (b c)", a=2)
                        nc.vector.tensor_scalar_add(out=ve,
                                                    in0=mv_sb[:, :, 1:2].rearrange("p (a b) c -> p a (b c)", a=2),
                                                    scalar1=LN_EPS)
                    nc.vector.reciprocal_approx_fast(out=y, in_=ve)
                    nc.vector.tensor_scalar(out=y, in0=y, scalar1=0.5,
                                            scalar2=0.5,
                                            op0=mybir.AluOpType.mult,
                                            op1=mybir.AluOpType.add)
                    for _ in range(O["newton_iters"]):
                        nc.vector.tensor_mul(out=tmp, in0=y, in1=y)
                        nc.vector.tensor_mul(out=tmp, in0=tmp, in1=ve)
                        nc.vector.tensor_scalar(out=tmp, in0=tmp, scalar1=-0.5,
                                                scalar2=1.5,
                                                op0=mybir.AluOpType.mult,
                                                op1=mybir.AluOpType.add)
                        nc.vector.tensor_mul(out=y, in0=y, in1=tmp)
                    nc.vector.scalar_tensor_tensor(
                        out=negms, in0=mu_view, scalar=-1.0,
                        in1=y, op0=mybir.AluOpType.mult,
                        op1=mybir.AluOpType.mult)
                    return (pes, y, negms)

                def backB(ch, pes, y, negms):
                    """ACT affine + bf16 store for chunk ch."""
                    if ch % 4 == 0:
                        out4_sb = work.tile([128, 16, D], BF16, tag="out4")
                        if LV < 7 or pes is None:
                            nc.gpsimd.memset(out4_sb, 0.0)
                        dma_cache["out4"] = out4_sb
                    out_sb = dma_cache["out4"][:, (ch % 4) * 4:(ch % 4) * 4 + 4, :]
                    if LV >= 7 and pes is not None:
                        ilv = O["ilv_stats"]
                        for g2 in range(2):
                            for jj in range(2):
                                g = g2 * 2 + jj
                                src_ap = (pes[g2][:, :, jj] if ilv
                                          else pes[g2][:, jj, :])
                                nc.scalar.activation(
                                    out=out_sb[:, g, :], in_=src_ap,
                                    func=AF.Identity,
                                    bias=negms[:, g2, jj:jj + 1],
                                    scale=y[:, g2, jj:jj + 1])
                                if apply_gb:
                                    nc.vector.tensor_mul(out=out_sb[:, g, :],
                                                         in0=out_sb[:, g, :],
                                                         in1=gb_sb[:, 0, :])
                                    nc.vector.tensor_add(out=out_sb[:, g, :],
                                                         in0=out_sb[:, g, :],
                                                         in1=gb_sb[:, 1, :])
                    if ch % 4 == 3:
                        nc.sync.dma_start(out=outv4[ch // 4],
                                          in_=dma_cache["out4"])

                if O["skew"] == 2:
                    enh_prev = None
                    A_prev = None
                    for ch in range(nch + 2):
                        enh_cur = front_half(ch) if ch < nch else None
                        if 0 <= ch - 1 < nch:
                            A_cur = backA(ch - 1,
                                          enh_prev[0] if enh_prev else None)
                        else:
                            A_cur = None
                        if 0 <= ch - 2 < nch:
                            backB(ch - 2, *(A_prev if A_prev
                                            else (None, None, None)))
                        enh_prev = enh_cur
                        A_prev = A_cur
                else:
                    enh_prev = None
                    for ch in range(nch + 1):
                        enh_cur = front_half(ch) if ch < nch else None
                        if 0 <= ch - 1 < nch:
                            A = backA(ch - 1,
                                      enh_prev[0] if enh_prev else None)
                            backB(ch - 1, *(A if A else (None, None, None)))
                        enh_prev = enh_cur

            if reps == 1:
                body()
            else:
                with tc.For_i(0, reps, 1,
                              hint_engines=(ET.PE, ET.DVE, ET.Activation,
                                            ET.Pool, ET.SP)):
                    body()

    nc.compile()
    return nc


_NC_CACHE = {}


def _get_nc(npc, apply_gb, reps=1, opts=None):
    key = (npc, apply_gb, reps, tuple(sorted((opts or {}).items())))
    if key not in _NC_CACHE:
        _NC_CACHE[key] = _build(npc, apply_gb, reps, opts=opts)
    return _NC_CACHE[key]


def _make_in_maps(node_feat, text_feat, segment_ids, W1, b1, W2, b2, Wg, bg,
                  ln_gamma, ln_beta):
    import ml_dtypes
    BF = ml_dtypes.bfloat16
    total = node_feat.shape[0]
    npc = total // N_CORES
    nodeT = np.ascontiguousarray(
        np.asarray(node_feat, np.float32).astype(BF).T)   # [256, total] bf16
    seg = np.asarray(segment_ids)
    onehot = (seg[None, :] == np.arange(B, dtype=seg.dtype)[:, None]
              ).astype(BF)                                # [64, total] bf16
    Wg = np.asarray(Wg, np.float32)
    shared = {
        "textT": np.ascontiguousarray(np.asarray(text_feat, np.float32).T
                                      ).astype(BF),
        "w1": np.asarray(W1, np.float32).astype(BF),
        "b1": np.asarray(b1, np.float32).astype(BF).reshape(1, HD),
        "w2": np.asarray(W2, np.float32).astype(BF),
        "b2": np.asarray(b2, np.float32).astype(BF).reshape(1, D),
        "wg1": np.ascontiguousarray(Wg[:D]).astype(BF),
        "wg2": np.ascontiguousarray(Wg[D:]).astype(BF),
        "bg": np.asarray(bg, np.float32).astype(BF).reshape(1, D),
        "gamma": np.asarray(ln_gamma, np.float32).reshape(1, D),
        "beta": np.asarray(ln_beta, np.float32).reshape(1, D),
        "onesd": np.ones((1, B), BF),
    }
    in_maps = []
    for c in range(N_CORES):
        m = dict(shared)
        m["nodeT"] = np.ascontiguousarray(nodeT[:, c * npc:(c + 1) * npc])
        m["onehot"] = np.ascontiguousarray(onehot[:, c * npc:(c + 1) * npc])
        in_maps.append(m)
    return in_maps, npc


def kernel(node_feat, text_feat, segment_ids, W1, b1, W2, b2, Wg, bg,
           ln_gamma, ln_beta):
    total, d = node_feat.shape
    npc = total // N_CORES
    assert npc % CHUNK == 0

    apply_gb = not (np.all(np.asarray(ln_gamma) == 1.0)
                    and np.all(np.asarray(ln_beta) == 0.0))
    in_maps, npc = _make_in_maps(node_feat, text_feat, segment_ids, W1, b1,
                                 W2, b2, Wg, bg, ln_gamma, ln_beta)
    nc = _get_nc(npc, apply_gb, 1)
    res = run_bass_kernel_spmd(nc, in_maps, core_ids=list(range(N_CORES)))
    out = np.concatenate([res.results[c]["out"] for c in range(N_CORES)], axis=0)
    return out.astype(np.float32)


def _make_sharded(nc):
    import jax
    from jax.experimental.shard_map import shard_map
    from jax.sharding import Mesh, PartitionSpec
    import concourse.bass2jax as b2j
    import concourse.mybir as mb

    b2j.install_neuronx_cc_hook()
    partition_name = (nc.partition_id_tensor.name
                      if nc.partition_id_tensor else None)
    in_names, out_names, out_avals = [], [], []
    for alloc in nc.m.functions[0].allocations:
        if not isinstance(alloc, mb.MemoryLocationSet):
            continue
        name = alloc.memorylocations[0].name
        if alloc.kind == "ExternalInput":
            if name != partition_name:
                in_names.append(name)
        elif alloc.kind == "ExternalOutput":
            out_names.append(name)
            out_avals.append(jax.core.ShapedArray(tuple(alloc.tensor_shape),
                                                  mb.dt.np(alloc.dtype)))
    n_params = len(in_names)
    n_outs = len(out_avals)
    in_names_all = list(in_names) + out_names
    if partition_name is not None:
        in_names_all.append(partition_name)
    donate = tuple(range(n_params, n_params + n_outs))

    def _body(*args):
        operands = list(args)
        if partition_name is not None:
            operands.append(b2j.partition_id_tensor())
        outs = b2j._bass_exec_p.bind(
            *operands, out_avals=tuple(out_avals), in_names=tuple(in_names_all),
            out_names=tuple(out_names), lowering_input_output_aliases=(),
            sim_require_finite=True, sim_require_nnan=True, nc=nc)
        return tuple(outs)

    devices = jax.devices()[:N_CORES]
    mesh = Mesh(np.asarray(devices), ("core",))
    sharded = jax.jit(
        shard_map(_body, mesh=mesh,
                  in_specs=(PartitionSpec("core"),) * (n_params + n_outs),
                  out_specs=(PartitionSpec("core"),) * n_outs,
                  check_rep=False),
        donate_argnums=donate, keep_unused=True)
    return sharded, in_names, out_avals


def _bench_walls(nc, in_maps, iters):
    import time
    import jax
    from jax.sharding import Mesh, PartitionSpec

    sharded, in_names, out_avals = _make_sharded(nc)
    mesh = Mesh(np.asarray(jax.devices()[:N_CORES]), ("core",))
    sh = jax.sharding.NamedSharding(mesh, PartitionSpec("core"))
    concat_in = [
        np.concatenate([np.asarray(in_maps[c][nm]) for c in range(N_CORES)],
                       axis=0)
        for nm in in_names]
    in_dev = [jax.device_put(a, sh) for a in concat_in]
    outs = [jax.device_put(
        np.zeros((N_CORES * z.shape[0], *z.shape[1:]), z.dtype), sh)
        for z in out_avals]
    jax.block_until_ready(in_dev + outs)
    outs = sharded(*in_dev, *outs)
    jax.block_until_ready(outs)
    times = []
    for _ in range(iters):
        t0 = time.perf_counter()
        outs = sharded(*in_dev, *outs)
        jax.block_until_ready(outs)
        times.append(time.perf_counter() - t0)
    times.sort()
    return times[len(times) // 2], times


def bench_hw(inputs, r1=16, r2=1040, iters=11):
    in_maps, npc = _make_in_maps(**inputs)
    apply_gb = not (np.all(np.asarray(inputs["ln_gamma"]) == 1.0)
                    and np.all(np.asarray(inputs["ln_beta"]) == 0.0))
    med1, t1 = _bench_walls(_get_nc(npc, apply_gb, r1), in_maps, iters)
    med2, t2 = _bench_walls(_get_nc(npc, apply_gb, r2), in_maps, iters)
    per_exec = (med2 - med1) / (r2 - r1)
    return per_exec, {"r1": r1, "r2": r2, "med1": med1, "med2": med2,
                      "times1": t1, "times2": t2}
